# revision 4
# baseline (speedup 1.0000x reference)
"""Trainium2 Bass kernel for nn_Embedding2Score (segment_reduce).

Strategy (data-parallel over sessions, per sharding hint):
  - 4096 graphs -> 8 cores x 512 graphs (4 blocks of 128 graphs each).
    Each core owns whole contiguous segments (batch is sorted by graph).
  - Nodes are processed in 512-node supertiles (4 x 128-node subtiles).
    Segment broadcast (v_n -> nodes) and segment sum (alpha*x -> s_g) are
    one-hot matmuls on PE; one-hot blocks are built with DVE is_equal.
    The graph-major one-hot S^T needs batchloc replicated down the 128
    partitions: done with a K=1 ones-matmul into PSUM (no DMA broadcast).
  - bf16 data path: x (both layouts), item_weight, W2, q_w and all one-hot
    matrices are bf16 (PE runs 1 cycle/row vs 4 for fp32, DMA bytes halve).
    Accumulation stays fp32 in PSUM; biases/alpha/v_n/W1/W3 stay fp32.
    rel-err budget is 2e-2; bf16 keeps it ~2e-3.
  - Final scoring: s_h^T [128d, 512g] per core vs item_weight^T tiles,
    grouped 4 vocab-tiles per DMA so all phase-2 DMAs are ~1 MB.
    Output rows [512, V] per core = row-slice of [4096, 50000].
"""

import sys

if "/opt/trn_rl_repo" not in sys.path:
    sys.path.insert(0, "/opt/trn_rl_repo")

import numpy as np
import ml_dtypes

BF16 = ml_dtypes.bfloat16
P = 128          # partitions / tile edge
D = 128          # hidden size
NCORES = 8
NBLK = 4         # graph blocks per core, 128 graphs each
BC = NBLK * P    # graphs per core = 512
VT = 500         # vocab tile (psum bank holds 512 fp32)
VG = 4           # vocab tiles per DMA group
ST = 4           # 128-node subtiles per supertile
NPF = 25         # vocab groups prefetched (all SBUF-resident)


def build_nc(ntpb, vpad, repeat=1, phase="both"):
    """Build the per-core Bass program. ntpb = node tiles per graph-block,
    vpad = padded vocab size (multiple of VT*VG). repeat>1 wraps the body
    in a hardware loop (timing probes). phase: 'both' | 'p1' | 'p2'."""
    import contextlib
    import concourse.bacc as bacc
    import concourse.mybir as mybir
    from concourse.tile import TileContext

    f32 = mybir.dt.float32
    bf16 = mybir.dt.bfloat16
    npb = ntpb * P
    nc = bacc.Bacc()

    xpk_ext = nc.declare_dram_parameter("xpk", [P, NBLK * npb], bf16, isOutput=False)
    xtp_ext = nc.declare_dram_parameter("xtp", [P, NBLK * npb], bf16, isOutput=False)
    blc_ext = nc.declare_dram_parameter("blc", [NBLK, P, ntpb], f32, isOutput=False)
    blr_ext = nc.declare_dram_parameter("blr", [NBLK, npb], bf16, isOutput=False)
    vnt_ext = nc.declare_dram_parameter("vnt", [D, BC], f32, isOutput=False)
    w1t_ext = nc.declare_dram_parameter("w1t", [D, D], f32, isOutput=False)
    w2t_ext = nc.declare_dram_parameter("w2t", [D, D], bf16, isOutput=False)
    w3at_ext = nc.declare_dram_parameter("w3at", [D, D], f32, isOutput=False)
    w3bt_ext = nc.declare_dram_parameter("w3bt", [D, D], f32, isOutput=False)
    b12c_ext = nc.declare_dram_parameter("b12c", [P, 1], f32, isOutput=False)
    w3bc_ext = nc.declare_dram_parameter("w3bc", [P, 1], f32, isOutput=False)
    qwt_ext = nc.declare_dram_parameter("qwt", [D, 1], bf16, isOutput=False)
    qbc_ext = nc.declare_dram_parameter("qbc", [P, 1], f32, isOutput=False)
    itwt_ext = nc.declare_dram_parameter("itwt", [D, vpad], bf16, isOutput=False)
    y_ext = nc.declare_dram_parameter("y", [BC, vpad], f32, isOutput=True)

    with TileContext(nc) as tc:
        with tc.tile_pool(name="const", bufs=1) as cp:
            iota_i = cp.tile([P, P], mybir.dt.int32, tag="iotai")
            nc.gpsimd.iota(iota_i[:], pattern=[[1, P]], base=0, channel_multiplier=0)
            iota_f = cp.tile([P, P], f32, tag="iotaf")
            nc.vector.tensor_copy(out=iota_f[:], in_=iota_i[:])
            iota_row = cp.tile([P, P], bf16, tag="iotarow")
            nc.vector.tensor_copy(out=iota_row[:], in_=iota_f[:])
            iota_ci = cp.tile([P, 1], mybir.dt.int32, tag="iotaci")
            nc.gpsimd.iota(iota_ci[:], pattern=[[0, 1]], base=0, channel_multiplier=1)
            iota_col = cp.tile([P, 1], f32, tag="iotacol")
            nc.vector.tensor_copy(out=iota_col[:], in_=iota_ci[:])
            ones_row = cp.tile([1, P], bf16, tag="onesrow")
            nc.vector.memset(ones_row[:], 1.0)

            def load(name, ext, shape, dt):
                t = cp.tile(shape, dt, tag=name)
                nc.sync.dma_start(out=t[:], in_=ext[:])
                return t

            w1t = load("w1t", w1t_ext, [D, D], f32)
            w2t = load("w2t", w2t_ext, [D, D], bf16)
            w3at = load("w3at", w3at_ext, [D, D], f32)
            w3bt = load("w3bt", w3bt_ext, [D, D], f32)
            b12c = load("b12c", b12c_ext, [P, 1], f32)
            w3bc = load("w3bc", w3bc_ext, [P, 1], f32)
            qwt = load("qwt", qwt_ext, [D, 1], bf16)
            qbc = load("qbc", qbc_ext, [P, 1], f32)
            vnt = load("vnt", vnt_ext, [D, BC], f32)

            shT = cp.tile([D, BC], bf16, tag="shT")  # s_h^T, filled per block
            if phase == "p2":
                nc.vector.memset(shT[:], 0.01)
            itw_pre = []
            W = VG * VT
            for g in range(min(NPF, vpad // W)):
                t = cp.tile([D, W], bf16, tag=f"itwpre{g}")
                nc.sync.dma_start(out=t[:], in_=itwt_ext[:, g * W:(g + 1) * W])
                itw_pre.append(t)

            rep_ctx = tc.For_i(0, repeat, 1) if repeat > 1 else contextlib.nullcontext()
            with rep_ctx:
                _build_body(nc, tc, mybir, ntpb, vpad,
                            xpk_ext, xtp_ext, blc_ext, blr_ext, itwt_ext, y_ext,
                            iota_row, iota_col, ones_row,
                            w1t, w2t, w3at, w3bt, b12c, w3bc, qwt, qbc, vnt, shT,
                            phase, itw_pre)

    nc.compile()
    return nc


def _build_body(nc, tc, mybir, ntpb, vpad,
                xpk_ext, xtp_ext, blc_ext, blr_ext, itwt_ext, y_ext,
                iota_row, iota_col, ones_row,
                w1t, w2t, w3at, w3bt, b12c, w3bc, qwt, qbc, vnt, shT,
                phase="both", itw_pre=None):
    f32 = mybir.dt.float32
    bf16 = mybir.dt.bfloat16
    npb = ntpb * P
    nst = -(-ntpb // ST)          # supertiles per block
    Sig = mybir.ActivationFunctionType.Sigmoid
    EQ = mybir.AluOpType.is_equal

    if phase in ("both", "p1"):
        with tc.tile_pool(name="p1big", bufs=2) as pb, \
             tc.tile_pool(name="p1", bufs=4) as pool, \
             tc.tile_pool(name="blkp", bufs=2) as blkp, \
             tc.tile_pool(name="psPre", bufs=2, space="PSUM") as psPre, \
             tc.tile_pool(name="psBc", bufs=2, space="PSUM") as psBc, \
             tc.tile_pool(name="psBlk", bufs=1, space="PSUM") as psBlk, \
             tc.tile_pool(name="psAl", bufs=1, space="PSUM") as psAl, \
             tc.tile_pool(name="psSg", bufs=1, space="PSUM") as psSg:
            for blk in range(NBLK):
                gsl = slice(blk * P, (blk + 1) * P)
                nsl = slice(blk * npb, (blk + 1) * npb)
                xpk = pb.tile([P, npb], bf16, tag="xpk")
                nc.sync.dma_start(out=xpk[:], in_=xpk_ext[:, nsl])
                xtp = pb.tile([P, npb], bf16, tag="xtp")
                nc.sync.dma_start(out=xtp[:], in_=xtp_ext[:, nsl])
                blc = blkp.tile([P, ntpb], f32, tag="blc")
                nc.sync.dma_start(out=blc[:], in_=blc_ext[blk])
                blr = blkp.tile([1, npb], bf16, tag="blr")
                nc.sync.dma_start(out=blr[:], in_=blr_ext[blk:blk + 1, :])
                # q1g[g, d] = (v_n_blk @ W1_w.T)[g, d]  (biases folded later)
                q1g_ps = psBlk.tile([P, P], f32, tag="blkmm", space="PSUM")
                nc.tensor.matmul(out=q1g_ps[:], lhsT=vnt[:, gsl], rhs=w1t[:],
                                 start=True, stop=True)
                q1g = blkp.tile([P, P], bf16, tag="q1g")
                nc.vector.tensor_copy(out=q1g[:], in_=q1g_ps[:])

                sg_ps = psSg.tile([P, P], f32, tag="sg", space="PSUM")
                mm_i = 0
                n_mm = sum(min(ST, ntpb - ST * s) for s in range(nst))
                for st in range(nst):
                    nsub = min(ST, ntpb - ST * st)
                    w = nsub * P
                    ssl = slice(st * ST * P, st * ST * P + w)  # cols in block
                    # batchloc replicated down partitions via K=1 ones-matmul
                    bc_ps = psBc.tile([P, ST * P], f32, tag="bc", space="PSUM")
                    nc.tensor.matmul(out=bc_ps[:, :w], lhsT=ones_row[:],
                                     rhs=blr[:, ssl], start=True, stop=True)
                    # S^T[g, n] = (batchloc[n] == g)   [one op, 512 wide]
                    StT = pool.tile([P, ST * P], bf16, tag="StT")
                    nc.vector.tensor_scalar(out=StT[:, :w], in0=bc_ps[:, :w],
                                            scalar1=iota_col[:], scalar2=None,
                                            op0=EQ)
                    # S[n, g] per 128-node subtile
                    S_st = pool.tile([P, ST * P], bf16, tag="S")
                    for c in range(nsub):
                        csl = slice(c * P, (c + 1) * P)
                        nc.gpsimd.tensor_scalar(
                            out=S_st[:, csl], in0=iota_row[:],
                            scalar1=blc[:, st * ST + c:st * ST + c + 1],
                            scalar2=None, op0=EQ)
                    # pre^T[d, n] = W2 @ x^T + q1g^T-expand   (+b12 in ACT)
                    pre_ps = psPre.tile([P, ST * P], f32, tag="pre", space="PSUM")
                    nc.tensor.matmul(out=pre_ps[:, :w], lhsT=w2t[:],
                                     rhs=xtp[:, ssl], start=True, stop=False)
                    nc.tensor.matmul(out=pre_ps[:, :w], lhsT=q1g[:],
                                     rhs=StT[:, :w], start=False, stop=True)
                    sigT = pool.tile([P, ST * P], bf16, tag="sigT")
                    nc.scalar.activation(out=sigT[:, :w], in_=pre_ps[:, :w],
                                         func=Sig, bias=b12c[:])
                    # alpha[n] = sig @ q_w.T (+ q_b in copy)
                    al_ps = psAl.tile([P, ST], f32, tag="al", space="PSUM")
                    for c in range(nsub):
                        csl = slice(c * P, (c + 1) * P)
                        nc.tensor.matmul(out=al_ps[:, c:c + 1],
                                         lhsT=sigT[:, csl], rhs=qwt[:],
                                         start=True, stop=True)
                    al = pool.tile([P, ST], f32, tag="al")
                    nc.vector.tensor_scalar_add(out=al[:, :nsub],
                                                in0=al_ps[:, :nsub],
                                                scalar1=qbc[:])
                    # xa = alpha * x ; s_g^T[d, g] += xa^T-reduce via S
                    xa = pool.tile([P, ST * P], bf16, tag="xa")
                    for c in range(nsub):
                        csl = slice(c * P, (c + 1) * P)
                        nc.vector.tensor_scalar_mul(
                            out=xa[:, csl],
                            in0=xpk[:, st * ST * P + c * P:st * ST * P + (c + 1) * P],
                            scalar1=al[:, c:c + 1])
                        nc.tensor.matmul(out=sg_ps[:], lhsT=xa[:, csl],
                                         rhs=S_st[:, csl],
                                         start=(mm_i == 0), stop=(mm_i == n_mm - 1))
                        mm_i += 1

                sg_sb = blkp.tile([P, P], f32, tag="sgsb")
                nc.vector.tensor_copy(out=sg_sb[:], in_=sg_ps[:])
                # s_h^T[d, g] = W3a @ v_n^T + W3b @ s_g^T  (+W3_b in copy)
                sh_ps = psBlk.tile([P, P], f32, tag="blkmm", space="PSUM")
                nc.tensor.matmul(out=sh_ps[:], lhsT=w3at[:], rhs=vnt[:, gsl],
                                 start=True, stop=False)
                nc.tensor.matmul(out=sh_ps[:], lhsT=w3bt[:], rhs=sg_sb[:],
                                 start=False, stop=True)
                nc.vector.tensor_scalar_add(out=shT[:, gsl], in0=sh_ps[:],
                                            scalar1=w3bc[:])

    if phase in ("both", "p2"):
        with tc.tile_pool(name="p2", bufs=2) as p2, \
             tc.tile_pool(name="p2o", bufs=6) as p2o, \
             tc.tile_pool(name="ps2", bufs=4, space="PSUM") as ps2:
            # ------------- phase 2: scores = s_h @ item_weight^T ----------
            W = VG * VT
            ngrp = vpad // W
            for g in range(ngrp):
                gvsl = slice(g * W, (g + 1) * W)
                if itw_pre is not None and g < len(itw_pre):
                    itw = itw_pre[g]
                else:
                    itw = p2.tile([D, W], bf16, tag="itw")
                    nc.sync.dma_start(out=itw[:], in_=itwt_ext[:, gvsl])
                for blk in range(NBLK):
                    gsl = slice(blk * P, (blk + 1) * P)
                    sc = p2o.tile([P, W], f32, tag="scsb")
                    for s in range(VG):
                        sc_ps = ps2.tile([P, VT], f32, tag="sc", space="PSUM")
                        nc.tensor.matmul(out=sc_ps[:],
                                         lhsT=shT[:, gsl],
                                         rhs=itw[:, s * VT:(s + 1) * VT],
                                         start=True, stop=True)
                        if s % 2 == 0:
                            nc.vector.tensor_copy(out=sc[:, s * VT:(s + 1) * VT],
                                                  in_=sc_ps[:])
                        else:
                            nc.scalar.copy(out=sc[:, s * VT:(s + 1) * VT],
                                           in_=sc_ps[:])
                    nc.sync.dma_start(out=y_ext[blk * P:(blk + 1) * P, gvsl],
                                      in_=sc[:])


def prep_inputs(session_embedding, item_weight, W1_w, W1_b, W2_w, W2_b,
                q_w, q_b, W3_w, W3_b, batch, num_graphs):
    """Host-side sharding/layout. Returns (in_maps, ntpb, vpad, V, cc)."""
    x = np.ascontiguousarray(np.asarray(session_embedding, dtype=np.float32))
    itw = np.asarray(item_weight, dtype=np.float32)
    batch = np.asarray(batch).astype(np.int64)
    B = int(num_graphs)
    N, d = x.shape
    V = itw.shape[0]
    assert d == D and B == NCORES * BC, (d, B)

    counts = np.bincount(batch, minlength=B)
    assert counts.min() >= 1, "every graph must be non-empty"
    starts = np.zeros(B + 1, np.int64)
    np.cumsum(counts, out=starts[1:])
    assert starts[-1] == N
    last_idx = starts[1:] - 1
    v_n = x[last_idx]                                   # [B, D]

    blk_cnt = starts[P::P] - starts[:-P:P].reshape(-1)  # [B//P]
    ntpb = int(-(-blk_cnt.max() // P))                  # ceil
    npb = ntpb * P

    vpad = -(-V // (VT * VG)) * (VT * VG)
    itwT = np.zeros((D, vpad), BF16)
    itwT[:, :V] = itw.T.astype(BF16)

    w1t = np.ascontiguousarray(np.asarray(W1_w, np.float32).T)
    w2t = np.ascontiguousarray(np.asarray(W2_w, np.float32).T.astype(BF16))
    W3 = np.asarray(W3_w, np.float32)
    w3at = np.ascontiguousarray(W3[:, :D].T)
    w3bt = np.ascontiguousarray(W3[:, D:].T)
    b12c = (np.asarray(W1_b, np.float32) + np.asarray(W2_b, np.float32)
            ).reshape(P, 1).copy()
    w3bc = np.asarray(W3_b, np.float32).reshape(P, 1).copy()
    qwt = np.ascontiguousarray(
        np.asarray(q_w, np.float32).reshape(1, D).T.astype(BF16))
    qbc = np.full((P, 1), np.float32(np.asarray(q_b).reshape(())), np.float32)

    in_maps = []
    for c in range(NCORES):
        xpad = np.zeros((NBLK, npb, D), np.float32)
        bl = np.zeros((NBLK, P, ntpb), np.float32)
        blr = np.zeros((NBLK, npb), BF16)
        for b in range(NBLK):
            glo = c * BC + b * P
            s, e = int(starts[glo]), int(starts[glo + P])
            cnt = e - s
            assert cnt <= npb
            xpad[b, :cnt] = x[s:e]
            locp = np.zeros(npb, np.float32)
            locp[:cnt] = (batch[s:e] - glo).astype(np.float32)
            bl[b] = locp.reshape(ntpb, P).T
            blr[b] = locp.astype(BF16)
        # packed node-row: xpk[:, blk*npb + t*128 + j][i] = x_pad[blk, t*128+i, j]
        xpk = np.ascontiguousarray(
            xpad.reshape(NBLK, ntpb, P, D).transpose(2, 0, 1, 3)
            .reshape(P, NBLK * npb).astype(BF16))
        # feature-row transposed: xtp[:, blk*npb + n] = x_pad[blk, n, :]
        xtp = np.ascontiguousarray(
            xpad.transpose(2, 0, 1).reshape(P, NBLK * npb).astype(BF16))
        vnt = np.ascontiguousarray(v_n[c * BC:(c + 1) * BC].T)
        im = dict(
            xpk=xpk, xtp=xtp, blc=np.ascontiguousarray(bl),
            blr=np.ascontiguousarray(blr), vnt=vnt,
            w1t=w1t, w2t=w2t, w3at=w3at, w3bt=w3bt,
            b12c=b12c, w3bc=w3bc, qwt=qwt, qbc=qbc, itwt=itwT)
        in_maps.append(im)
    return in_maps, ntpb, vpad, V, False


_NC_CACHE = {}


def get_nc(ntpb, vpad, repeat=1, phase="both", cc=False):
    key = (ntpb, vpad, repeat, phase)
    if key not in _NC_CACHE:
        _NC_CACHE[key] = build_nc(ntpb, vpad, repeat, phase)
    return _NC_CACHE[key]


def kernel(**inputs):
    from concourse.bass_utils import run_bass_kernel_spmd

    in_maps, ntpb, vpad, V, cc = prep_inputs(**inputs)
    nc = get_nc(ntpb, vpad)
    res = run_bass_kernel_spmd(nc, in_maps, core_ids=list(range(NCORES)))
    B = int(inputs["num_graphs"])
    y = np.empty((B, V), np.float32)
    for c in range(NCORES):
        y[c * BC:(c + 1) * BC] = res.results[c]["y"][:, :V]
    return y


# revision 5
# speedup vs baseline: 1.5242x; 1.5242x over previous
"""Trainium2 Bass kernel for nn_Embedding2Score (segment_reduce).

Strategy (data-parallel over sessions, per sharding hint):
  - 4096 graphs -> 8 cores x 512 graphs (4 blocks of 128 graphs each).
    Each core owns whole contiguous segments (batch is sorted by graph).
  - Nodes are processed in 512-node supertiles (4 x 128-node subtiles).
    Segment broadcast (v_n -> nodes) and segment sum (alpha*x -> s_g) are
    one-hot matmuls on PE; one-hot blocks are built with DVE is_equal.
    The graph-major one-hot S^T needs batchloc replicated down the 128
    partitions: done with a K=1 ones-matmul into PSUM (no DMA broadcast).
  - bf16 data path: x (both layouts), item_weight, W2, q_w and all one-hot
    matrices are bf16 (PE runs 1 cycle/row vs 4 for fp32, DMA bytes halve).
    Accumulation stays fp32 in PSUM; biases/alpha/v_n/W1/W3 stay fp32.
    rel-err budget is 2e-2; bf16 keeps it ~2e-3.
  - Final scoring: s_h^T [128d, 512g] per core vs item_weight^T tiles,
    grouped 4 vocab-tiles per DMA so all phase-2 DMAs are ~1 MB.
    Output rows [512, V] per core = row-slice of [4096, 50000].
"""

import sys

if "/opt/trn_rl_repo" not in sys.path:
    sys.path.insert(0, "/opt/trn_rl_repo")

import numpy as np
import ml_dtypes

BF16 = ml_dtypes.bfloat16
P = 128          # partitions / tile edge
D = 128          # hidden size
NCORES = 8
NBLK = 4         # graph blocks per core, 128 graphs each
BC = NBLK * P    # graphs per core = 512
VT = 500         # vocab tile (psum bank holds 512 fp32)
VG = 4           # vocab tiles per DMA group
ST = 4           # 128-node subtiles per supertile
NPF = 25         # vocab groups prefetched (all SBUF-resident)


def build_nc(ntpb, vpad, repeat=1, phase="both"):
    """Build the per-core Bass program. ntpb = node tiles per graph-block,
    vpad = padded vocab size (multiple of VT*VG). repeat>1 wraps the body
    in a hardware loop (timing probes). phase: 'both' | 'p1' | 'p2'."""
    import contextlib
    import concourse.bacc as bacc
    import concourse.mybir as mybir
    from concourse.tile import TileContext

    f32 = mybir.dt.float32
    bf16 = mybir.dt.bfloat16
    npb = ntpb * P
    nc = bacc.Bacc()

    xpk_ext = nc.declare_dram_parameter("xpk", [P, NBLK * npb], bf16, isOutput=False)
    xtp_ext = nc.declare_dram_parameter("xtp", [P, NBLK * npb], bf16, isOutput=False)
    blc_ext = nc.declare_dram_parameter("blc", [NBLK, P, ntpb], f32, isOutput=False)
    blr_ext = nc.declare_dram_parameter("blr", [NBLK, npb], bf16, isOutput=False)
    vnt_ext = nc.declare_dram_parameter("vnt", [D, BC], f32, isOutput=False)
    w1t_ext = nc.declare_dram_parameter("w1t", [D, D], f32, isOutput=False)
    w2t_ext = nc.declare_dram_parameter("w2t", [D, D], bf16, isOutput=False)
    w3at_ext = nc.declare_dram_parameter("w3at", [D, D], f32, isOutput=False)
    w3bt_ext = nc.declare_dram_parameter("w3bt", [D, D], f32, isOutput=False)
    b12c_ext = nc.declare_dram_parameter("b12c", [P, 1], f32, isOutput=False)
    w3bc_ext = nc.declare_dram_parameter("w3bc", [P, 1], f32, isOutput=False)
    qwt_ext = nc.declare_dram_parameter("qwt", [D, 1], bf16, isOutput=False)
    qbc_ext = nc.declare_dram_parameter("qbc", [P, 1], f32, isOutput=False)
    itwt_ext = nc.declare_dram_parameter("itwt", [D, vpad], bf16, isOutput=False)
    y_ext = nc.declare_dram_parameter("y", [BC, vpad], f32, isOutput=True)

    with TileContext(nc) as tc:
        with tc.tile_pool(name="const", bufs=1) as cp:
            iota_i = cp.tile([P, P], mybir.dt.int32, tag="iotai")
            nc.gpsimd.iota(iota_i[:], pattern=[[1, P]], base=0, channel_multiplier=0)
            iota_f = cp.tile([P, P], f32, tag="iotaf")
            nc.vector.tensor_copy(out=iota_f[:], in_=iota_i[:])
            iota_row = cp.tile([P, P], bf16, tag="iotarow")
            nc.vector.tensor_copy(out=iota_row[:], in_=iota_f[:])
            iota_ci = cp.tile([P, 1], mybir.dt.int32, tag="iotaci")
            nc.gpsimd.iota(iota_ci[:], pattern=[[0, 1]], base=0, channel_multiplier=1)
            iota_col = cp.tile([P, 1], f32, tag="iotacol")
            nc.vector.tensor_copy(out=iota_col[:], in_=iota_ci[:])
            ones_row = cp.tile([1, P], bf16, tag="onesrow")
            nc.vector.memset(ones_row[:], 1.0)

            def load(name, ext, shape, dt):
                t = cp.tile(shape, dt, tag=name)
                nc.sync.dma_start(out=t[:], in_=ext[:])
                return t

            w1t = load("w1t", w1t_ext, [D, D], f32)
            w2t = load("w2t", w2t_ext, [D, D], bf16)
            w3at = load("w3at", w3at_ext, [D, D], f32)
            w3bt = load("w3bt", w3bt_ext, [D, D], f32)
            b12c = load("b12c", b12c_ext, [P, 1], f32)
            w3bc = load("w3bc", w3bc_ext, [P, 1], f32)
            qwt = load("qwt", qwt_ext, [D, 1], bf16)
            qbc = load("qbc", qbc_ext, [P, 1], f32)
            vnt = load("vnt", vnt_ext, [D, BC], f32)

            shT = cp.tile([D, BC], bf16, tag="shT")  # s_h^T, filled per block
            if phase == "p2":
                nc.vector.memset(shT[:], 0.01)
            itw_pre = []
            W = VG * VT
            for g in range(min(NPF, vpad // W)):
                t = cp.tile([D, W], bf16, tag=f"itwpre{g}")
                nc.sync.dma_start(out=t[:], in_=itwt_ext[:, g * W:(g + 1) * W])
                itw_pre.append(t)

            rep_ctx = tc.For_i(0, repeat, 1) if repeat > 1 else contextlib.nullcontext()
            with rep_ctx:
                _build_body(nc, tc, mybir, ntpb, vpad,
                            xpk_ext, xtp_ext, blc_ext, blr_ext, itwt_ext, y_ext,
                            iota_row, iota_col, ones_row,
                            w1t, w2t, w3at, w3bt, b12c, w3bc, qwt, qbc, vnt, shT,
                            phase, itw_pre)

    nc.compile()
    return nc


def _build_body(nc, tc, mybir, ntpb, vpad,
                xpk_ext, xtp_ext, blc_ext, blr_ext, itwt_ext, y_ext,
                iota_row, iota_col, ones_row,
                w1t, w2t, w3at, w3bt, b12c, w3bc, qwt, qbc, vnt, shT,
                phase="both", itw_pre=None):
    f32 = mybir.dt.float32
    bf16 = mybir.dt.bfloat16
    npb = ntpb * P
    nst = -(-ntpb // ST)          # supertiles per block
    Sig = mybir.ActivationFunctionType.Sigmoid
    EQ = mybir.AluOpType.is_equal

    if phase in ("both", "p1"):
        with tc.tile_pool(name="p1big", bufs=2) as pb, \
             tc.tile_pool(name="p1", bufs=4) as pool, \
             tc.tile_pool(name="blkp", bufs=2) as blkp, \
             tc.tile_pool(name="psPre", bufs=2, space="PSUM") as psPre, \
             tc.tile_pool(name="psBc", bufs=2, space="PSUM") as psBc, \
             tc.tile_pool(name="psBlk", bufs=1, space="PSUM") as psBlk, \
             tc.tile_pool(name="psAl", bufs=1, space="PSUM") as psAl, \
             tc.tile_pool(name="psSg", bufs=1, space="PSUM") as psSg:
            for blk in range(NBLK):
                gsl = slice(blk * P, (blk + 1) * P)
                nsl = slice(blk * npb, (blk + 1) * npb)
                xpk = pb.tile([P, npb], bf16, tag="xpk")
                nc.sync.dma_start(out=xpk[:], in_=xpk_ext[:, nsl])
                xtp = pb.tile([P, npb], bf16, tag="xtp")
                nc.sync.dma_start(out=xtp[:], in_=xtp_ext[:, nsl])
                blc = blkp.tile([P, ntpb], f32, tag="blc")
                nc.sync.dma_start(out=blc[:], in_=blc_ext[blk])
                blr = blkp.tile([1, npb], bf16, tag="blr")
                nc.sync.dma_start(out=blr[:], in_=blr_ext[blk:blk + 1, :])
                # q1g[g, d] = (v_n_blk @ W1_w.T)[g, d]  (biases folded later)
                q1g_ps = psBlk.tile([P, P], f32, tag="blkmm", space="PSUM")
                nc.tensor.matmul(out=q1g_ps[:], lhsT=vnt[:, gsl], rhs=w1t[:],
                                 start=True, stop=True)
                q1g = blkp.tile([P, P], bf16, tag="q1g")
                nc.vector.tensor_copy(out=q1g[:], in_=q1g_ps[:])

                sg_ps = psSg.tile([P, P], f32, tag="sg", space="PSUM")
                mm_i = 0
                n_mm = sum(min(ST, ntpb - ST * s) for s in range(nst))
                for st in range(nst):
                    nsub = min(ST, ntpb - ST * st)
                    w = nsub * P
                    ssl = slice(st * ST * P, st * ST * P + w)  # cols in block
                    # batchloc replicated down partitions via K=1 ones-matmul
                    bc_ps = psBc.tile([P, ST * P], f32, tag="bc", space="PSUM")
                    nc.tensor.matmul(out=bc_ps[:, :w], lhsT=ones_row[:],
                                     rhs=blr[:, ssl], start=True, stop=True)
                    # S^T[g, n] = (batchloc[n] == g)   [one op, 512 wide]
                    StT = pool.tile([P, ST * P], bf16, tag="StT")
                    nc.vector.tensor_scalar(out=StT[:, :w], in0=bc_ps[:, :w],
                                            scalar1=iota_col[:], scalar2=None,
                                            op0=EQ)
                    # S[n, g] per 128-node subtile
                    S_st = pool.tile([P, ST * P], bf16, tag="S")
                    for c in range(nsub):
                        csl = slice(c * P, (c + 1) * P)
                        nc.vector.tensor_scalar(
                            out=S_st[:, csl], in0=iota_row[:],
                            scalar1=blc[:, st * ST + c:st * ST + c + 1],
                            scalar2=None, op0=EQ)
                    # pre^T[d, n] = W2 @ x^T + q1g^T-expand   (+b12 in ACT)
                    pre_ps = psPre.tile([P, ST * P], f32, tag="pre", space="PSUM")
                    nc.tensor.matmul(out=pre_ps[:, :w], lhsT=w2t[:],
                                     rhs=xtp[:, ssl], start=True, stop=False)
                    nc.tensor.matmul(out=pre_ps[:, :w], lhsT=q1g[:],
                                     rhs=StT[:, :w], start=False, stop=True)
                    sigT = pool.tile([P, ST * P], bf16, tag="sigT")
                    nc.scalar.activation(out=sigT[:, :w], in_=pre_ps[:, :w],
                                         func=Sig, bias=b12c[:])
                    # alpha[n] = sig @ q_w.T (+ q_b in copy)
                    al_ps = psAl.tile([P, ST], f32, tag="al", space="PSUM")
                    for c in range(nsub):
                        csl = slice(c * P, (c + 1) * P)
                        nc.tensor.matmul(out=al_ps[:, c:c + 1],
                                         lhsT=sigT[:, csl], rhs=qwt[:],
                                         start=True, stop=True)
                    al = pool.tile([P, ST], f32, tag="al")
                    nc.vector.tensor_scalar_add(out=al[:, :nsub],
                                                in0=al_ps[:, :nsub],
                                                scalar1=qbc[:])
                    # xa = alpha * x ; s_g^T[d, g] += xa^T-reduce via S
                    xa = pool.tile([P, ST * P], bf16, tag="xa")
                    for c in range(nsub):
                        csl = slice(c * P, (c + 1) * P)
                        nc.vector.tensor_scalar_mul(
                            out=xa[:, csl],
                            in0=xpk[:, st * ST * P + c * P:st * ST * P + (c + 1) * P],
                            scalar1=al[:, c:c + 1])
                        nc.tensor.matmul(out=sg_ps[:], lhsT=xa[:, csl],
                                         rhs=S_st[:, csl],
                                         start=(mm_i == 0), stop=(mm_i == n_mm - 1))
                        mm_i += 1

                sg_sb = blkp.tile([P, P], f32, tag="sgsb")
                nc.vector.tensor_copy(out=sg_sb[:], in_=sg_ps[:])
                # s_h^T[d, g] = W3a @ v_n^T + W3b @ s_g^T  (+W3_b in copy)
                sh_ps = psBlk.tile([P, P], f32, tag="blkmm", space="PSUM")
                nc.tensor.matmul(out=sh_ps[:], lhsT=w3at[:], rhs=vnt[:, gsl],
                                 start=True, stop=False)
                nc.tensor.matmul(out=sh_ps[:], lhsT=w3bt[:], rhs=sg_sb[:],
                                 start=False, stop=True)
                nc.vector.tensor_scalar_add(out=shT[:, gsl], in0=sh_ps[:],
                                            scalar1=w3bc[:])

    if phase in ("both", "p2"):
        with tc.tile_pool(name="p2", bufs=2) as p2, \
             tc.tile_pool(name="p2o", bufs=6) as p2o, \
             tc.tile_pool(name="ps2", bufs=4, space="PSUM") as ps2:
            # ------------- phase 2: scores = s_h @ item_weight^T ----------
            W = VG * VT
            ngrp = vpad // W
            for g in range(ngrp):
                gvsl = slice(g * W, (g + 1) * W)
                if itw_pre is not None and g < len(itw_pre):
                    itw = itw_pre[g]
                else:
                    itw = p2.tile([D, W], bf16, tag="itw")
                    nc.sync.dma_start(out=itw[:], in_=itwt_ext[:, gvsl])
                for blk in range(NBLK):
                    gsl = slice(blk * P, (blk + 1) * P)
                    sc = p2o.tile([P, W], f32, tag="scsb")
                    for s in range(VG):
                        sc_ps = ps2.tile([P, VT], f32, tag="sc", space="PSUM")
                        nc.tensor.matmul(out=sc_ps[:],
                                         lhsT=shT[:, gsl],
                                         rhs=itw[:, s * VT:(s + 1) * VT],
                                         start=True, stop=True)
                        if s % 2 == 0:
                            nc.vector.tensor_copy(out=sc[:, s * VT:(s + 1) * VT],
                                                  in_=sc_ps[:])
                        else:
                            nc.scalar.copy(out=sc[:, s * VT:(s + 1) * VT],
                                           in_=sc_ps[:])
                    nc.sync.dma_start(out=y_ext[blk * P:(blk + 1) * P, gvsl],
                                      in_=sc[:])


def prep_inputs(session_embedding, item_weight, W1_w, W1_b, W2_w, W2_b,
                q_w, q_b, W3_w, W3_b, batch, num_graphs):
    """Host-side sharding/layout. Returns (in_maps, ntpb, vpad, V, cc)."""
    x = np.ascontiguousarray(np.asarray(session_embedding, dtype=np.float32))
    itw = np.asarray(item_weight, dtype=np.float32)
    batch = np.asarray(batch).astype(np.int64)
    B = int(num_graphs)
    N, d = x.shape
    V = itw.shape[0]
    assert d == D and B == NCORES * BC, (d, B)

    counts = np.bincount(batch, minlength=B)
    assert counts.min() >= 1, "every graph must be non-empty"
    starts = np.zeros(B + 1, np.int64)
    np.cumsum(counts, out=starts[1:])
    assert starts[-1] == N
    last_idx = starts[1:] - 1
    v_n = x[last_idx]                                   # [B, D]

    blk_cnt = starts[P::P] - starts[:-P:P].reshape(-1)  # [B//P]
    ntpb = int(-(-blk_cnt.max() // P))                  # ceil
    npb = ntpb * P

    vpad = -(-V // (VT * VG)) * (VT * VG)
    itwT = np.zeros((D, vpad), BF16)
    itwT[:, :V] = itw.T.astype(BF16)

    w1t = np.ascontiguousarray(np.asarray(W1_w, np.float32).T)
    w2t = np.ascontiguousarray(np.asarray(W2_w, np.float32).T.astype(BF16))
    W3 = np.asarray(W3_w, np.float32)
    w3at = np.ascontiguousarray(W3[:, :D].T)
    w3bt = np.ascontiguousarray(W3[:, D:].T)
    b12c = (np.asarray(W1_b, np.float32) + np.asarray(W2_b, np.float32)
            ).reshape(P, 1).copy()
    w3bc = np.asarray(W3_b, np.float32).reshape(P, 1).copy()
    qwt = np.ascontiguousarray(
        np.asarray(q_w, np.float32).reshape(1, D).T.astype(BF16))
    qbc = np.full((P, 1), np.float32(np.asarray(q_b).reshape(())), np.float32)

    in_maps = []
    for c in range(NCORES):
        xpad = np.zeros((NBLK, npb, D), np.float32)
        bl = np.zeros((NBLK, P, ntpb), np.float32)
        blr = np.zeros((NBLK, npb), BF16)
        for b in range(NBLK):
            glo = c * BC + b * P
            s, e = int(starts[glo]), int(starts[glo + P])
            cnt = e - s
            assert cnt <= npb
            xpad[b, :cnt] = x[s:e]
            locp = np.zeros(npb, np.float32)
            locp[:cnt] = (batch[s:e] - glo).astype(np.float32)
            bl[b] = locp.reshape(ntpb, P).T
            blr[b] = locp.astype(BF16)
        # packed node-row: xpk[:, blk*npb + t*128 + j][i] = x_pad[blk, t*128+i, j]
        xpk = np.ascontiguousarray(
            xpad.reshape(NBLK, ntpb, P, D).transpose(2, 0, 1, 3)
            .reshape(P, NBLK * npb).astype(BF16))
        # feature-row transposed: xtp[:, blk*npb + n] = x_pad[blk, n, :]
        xtp = np.ascontiguousarray(
            xpad.transpose(2, 0, 1).reshape(P, NBLK * npb).astype(BF16))
        vnt = np.ascontiguousarray(v_n[c * BC:(c + 1) * BC].T)
        im = dict(
            xpk=xpk, xtp=xtp, blc=np.ascontiguousarray(bl),
            blr=np.ascontiguousarray(blr), vnt=vnt,
            w1t=w1t, w2t=w2t, w3at=w3at, w3bt=w3bt,
            b12c=b12c, w3bc=w3bc, qwt=qwt, qbc=qbc, itwt=itwT)
        in_maps.append(im)
    return in_maps, ntpb, vpad, V, False


_NC_CACHE = {}


def get_nc(ntpb, vpad, repeat=1, phase="both", cc=False):
    key = (ntpb, vpad, repeat, phase)
    if key not in _NC_CACHE:
        _NC_CACHE[key] = build_nc(ntpb, vpad, repeat, phase)
    return _NC_CACHE[key]


def kernel(**inputs):
    from concourse.bass_utils import run_bass_kernel_spmd

    in_maps, ntpb, vpad, V, cc = prep_inputs(**inputs)
    nc = get_nc(ntpb, vpad)
    res = run_bass_kernel_spmd(nc, in_maps, core_ids=list(range(NCORES)))
    B = int(inputs["num_graphs"])
    y = np.empty((B, V), np.float32)
    for c in range(NCORES):
        y[c * BC:(c + 1) * BC] = res.results[c]["y"][:, :V]
    return y


# revision 15
# speedup vs baseline: 1.7458x; 1.1454x over previous
"""Trainium2 Bass kernel for nn_Embedding2Score (segment_reduce).

Strategy (data-parallel over sessions, per sharding hint):
  - 4096 graphs -> 8 cores x 512 graphs (4 blocks of 128 graphs each).
    Each core owns whole contiguous segments (batch is sorted by graph).
  - Nodes are processed in 512-node supertiles (4 x 128-node subtiles).
    Segment broadcast (v_n -> nodes) and segment sum (alpha*x -> s_g) are
    one-hot matmuls on PE; one-hot blocks are built with DVE is_equal.
    The graph-major one-hot S^T needs batchloc replicated down the 128
    partitions: done with a K=1 ones-matmul into PSUM (no DMA broadcast).
  - bf16 data path: x (both layouts), item_weight, W2, q_w and all one-hot
    matrices are bf16 (PE runs 1 cycle/row vs 4 for fp32, DMA bytes halve).
    Accumulation stays fp32 in PSUM; biases/alpha/v_n/W1/W3 stay fp32.
    rel-err budget is 2e-2; bf16 keeps it ~2e-3.
  - Final scoring: s_h^T [128d, 512g] per core vs item_weight^T tiles,
    grouped 4 vocab-tiles per DMA so all phase-2 DMAs are ~1 MB.
    Output rows [512, V] per core = row-slice of [4096, 50000].
"""

import sys

if "/opt/trn_rl_repo" not in sys.path:
    sys.path.insert(0, "/opt/trn_rl_repo")

import numpy as np
import ml_dtypes

BF16 = ml_dtypes.bfloat16
P = 128          # partitions / tile edge
D = 128          # hidden size
NCORES = 8
NBLK = 4         # graph blocks per core, 128 graphs each
BC = NBLK * P    # graphs per core = 512
VT = 500         # vocab tile (psum bank holds 512 fp32)
VG = 4           # vocab tiles per DMA group
ST = 4           # 128-node subtiles per supertile
NPF = 23         # vocab groups prefetched (SBUF-resident)


def build_nc(ntpb, vpad, repeat=1, phase="both"):
    """Build the per-core Bass program. ntpb = node tiles per graph-block,
    vpad = padded vocab size (multiple of VT*VG). repeat>1 wraps the body
    in a hardware loop (timing probes). phase: 'both' | 'p1' | 'p2'."""
    import contextlib
    import concourse.bacc as bacc
    import concourse.mybir as mybir
    from concourse.tile import TileContext

    f32 = mybir.dt.float32
    bf16 = mybir.dt.bfloat16
    npb = ntpb * P
    nc = bacc.Bacc()

    xpk_ext = nc.declare_dram_parameter("xpk", [P, NBLK * npb], bf16, isOutput=False)
    xtp_ext = nc.declare_dram_parameter("xtp", [P, NBLK * npb], bf16, isOutput=False)
    blc_ext = nc.declare_dram_parameter("blc", [NBLK, P, ntpb], f32, isOutput=False)
    blr_ext = nc.declare_dram_parameter("blr", [NBLK, npb], bf16, isOutput=False)
    vnt_ext = nc.declare_dram_parameter("vnt", [D, BC], f32, isOutput=False)
    w1t_ext = nc.declare_dram_parameter("w1t", [D, D], f32, isOutput=False)
    w2t_ext = nc.declare_dram_parameter("w2t", [D, D], bf16, isOutput=False)
    w3at_ext = nc.declare_dram_parameter("w3at", [D, D], f32, isOutput=False)
    w3bt_ext = nc.declare_dram_parameter("w3bt", [D, D], f32, isOutput=False)
    b12c_ext = nc.declare_dram_parameter("b12c", [P, 1], f32, isOutput=False)
    w3bc_ext = nc.declare_dram_parameter("w3bc", [P, 1], f32, isOutput=False)
    qwt_ext = nc.declare_dram_parameter("qwt", [D, 1], bf16, isOutput=False)
    sel4_ext = nc.declare_dram_parameter("sel4", [NBLK, NBLK * P], bf16,
                                         isOutput=False)
    qbc_ext = nc.declare_dram_parameter("qbc", [P, 1], f32, isOutput=False)
    itwt_ext = nc.declare_dram_parameter("itwt", [D, vpad], bf16, isOutput=False)
    y_ext = nc.declare_dram_parameter("y", [BC, vpad], f32, isOutput=True)

    with TileContext(nc) as tc:
        with tc.tile_pool(name="const", bufs=1) as cp:
            iota_i = cp.tile([P, P], mybir.dt.int32, tag="iotai")
            nc.gpsimd.iota(iota_i[:], pattern=[[1, P]], base=0, channel_multiplier=0)
            iota_f = cp.tile([P, P], f32, tag="iotaf")
            nc.vector.tensor_copy(out=iota_f[:], in_=iota_i[:])
            iota_row = cp.tile([P, P], bf16, tag="iotarow")
            nc.vector.tensor_copy(out=iota_row[:], in_=iota_f[:])
            iota_ci = cp.tile([P, 1], mybir.dt.int32, tag="iotaci")
            nc.gpsimd.iota(iota_ci[:], pattern=[[0, 1]], base=0, channel_multiplier=1)
            iota_col = cp.tile([P, 1], f32, tag="iotacol")
            nc.vector.tensor_copy(out=iota_col[:], in_=iota_ci[:])
            ones_row = cp.tile([1, P], bf16, tag="onesrow")
            nc.vector.memset(ones_row[:], 1.0)

            def load(name, ext, shape, dt):
                t = cp.tile(shape, dt, tag=name)
                nc.sync.dma_start(out=t[:], in_=ext[:])
                return t

            w1t = load("w1t", w1t_ext, [D, D], f32)
            w2t = load("w2t", w2t_ext, [D, D], bf16)
            w3at = load("w3at", w3at_ext, [D, D], f32)
            w3bt = load("w3bt", w3bt_ext, [D, D], f32)
            b12c = load("b12c", b12c_ext, [P, 1], f32)
            w3bc = load("w3bc", w3bc_ext, [P, 1], f32)
            qwt = load("qwt", qwt_ext, [D, 1], bf16)
            qbc = load("qbc", qbc_ext, [P, 1], f32)
            vnt = load("vnt", vnt_ext, [D, BC], f32)

            shT = cp.tile([D, BC], bf16, tag="shT")  # s_h^T, filled per block
            if phase == "p2":
                nc.vector.memset(shT[:], 0.01)
            # per-block batchloc tables are iteration-invariant: load once
            blc_sb = []
            for blk in range(NBLK):
                t = cp.tile([P, ntpb], f32, tag=f"blc{blk}")
                nc.sync.dma_start(out=t[:], in_=blc_ext[blk])
                blc_sb.append(t)
            blr4 = cp.tile([NBLK, npb], bf16, tag="blr4")
            nc.sync.dma_start(out=blr4[:], in_=blr_ext[:])
            sel4 = cp.tile([NBLK, NBLK * P], bf16, tag="sel4")
            nc.sync.dma_start(out=sel4[:], in_=sel4_ext[:])
            itw_pre = []
            W = VG * VT
            for g in range(min(NPF, vpad // W)):
                t = cp.tile([D, W], bf16, tag=f"itwpre{g}")
                nc.sync.dma_start(out=t[:], in_=itwt_ext[:, g * W:(g + 1) * W])
                itw_pre.append(t)

            rep_ctx = tc.For_i(0, repeat, 1) if repeat > 1 else contextlib.nullcontext()
            with rep_ctx:
                _build_body(nc, tc, mybir, ntpb, vpad,
                            xpk_ext, xtp_ext, blc_sb, blr4, sel4, itwt_ext, y_ext,
                            iota_row, iota_col, ones_row,
                            w1t, w2t, w3at, w3bt, b12c, w3bc, qwt, qbc, vnt, shT,
                            phase, itw_pre)

    nc.compile()
    return nc


def _build_body(nc, tc, mybir, ntpb, vpad,
                xpk_ext, xtp_ext, blc_sb, blr4, sel4, itwt_ext, y_ext,
                iota_row, iota_col, ones_row,
                w1t, w2t, w3at, w3bt, b12c, w3bc, qwt, qbc, vnt, shT,
                phase="both", itw_pre=None):
    f32 = mybir.dt.float32
    bf16 = mybir.dt.bfloat16
    npb = ntpb * P
    nst = -(-ntpb // ST)          # supertiles per block
    Sig = mybir.ActivationFunctionType.Sigmoid
    EQ = mybir.AluOpType.is_equal
    Wg = VG * VT
    ngrp = vpad // Wg
    do1 = phase in ("both", "p1")
    do2 = phase in ("both", "p2")

    # phase 1 (attention + segment reduce) and phase 2 (vocab scoring) are
    # interleaved per 128-graph block so the big y stores start ~1/4 of the
    # way into the iteration instead of after all of phase 1.
    with tc.tile_pool(name="p1big", bufs=2) as pb, \
         tc.tile_pool(name="p1", bufs=3) as pool, \
         tc.tile_pool(name="blkp", bufs=2) as blkp, \
         tc.tile_pool(name="p2o", bufs=6) as p2o, \
         tc.tile_pool(name="p2l", bufs=1) as p2l, \
         tc.tile_pool(name="psPre", bufs=2, space="PSUM") as psPre, \
         tc.tile_pool(name="psBc", bufs=1, space="PSUM") as psBc, \
         tc.tile_pool(name="psS", bufs=1, space="PSUM") as psS, \
         tc.tile_pool(name="psAl", bufs=1, space="PSUM") as psAl, \
         tc.tile_pool(name="ps2", bufs=3, space="PSUM") as ps2:
        for blk in range(NBLK):
            gsl = slice(blk * P, (blk + 1) * P)
            nsl = slice(blk * npb, (blk + 1) * npb)
            if do1:
                psS_t = psS.tile([P, 4 * P], f32, tag="psS", space="PSUM")
                xpk = pb.tile([P, npb], bf16, tag="xpk")
                nc.sync.dma_start(out=xpk[:], in_=xpk_ext[:, nsl])
                xtp = pb.tile([P, npb], bf16, tag="xtp")
                nc.sync.dma_start(out=xtp[:], in_=xtp_ext[:, nsl])
                blc = blc_sb[blk]
                # q1g[g, d] = (v_n_blk @ W1_w.T)[g, d]  (biases folded later)
                q1g_ps = psS_t[:, 0:P]
                nc.tensor.matmul(out=q1g_ps, lhsT=vnt[:, gsl], rhs=w1t[:],
                                 start=True, stop=True)
                q1g = blkp.tile([P, P], bf16, tag="q1g")
                nc.vector.tensor_copy(out=q1g[:], in_=q1g_ps)

                sg_ps = psS_t[:, P:2 * P]
                mm_i = 0
                n_mm = sum(min(ST, ntpb - ST * s) for s in range(nst))
                for st in range(nst):
                    nsub = min(ST, ntpb - ST * st)
                    w = nsub * P
                    ssl = slice(st * ST * P, st * ST * P + w)  # cols in block
                    # batchloc replicated down partitions via K=4 row-select
                    bc_ps = psBc.tile([P, ST * P], f32, tag="bc", space="PSUM")
                    nc.tensor.matmul(out=bc_ps[:, :w],
                                     lhsT=sel4[:, blk * P:(blk + 1) * P],
                                     rhs=blr4[:, ssl], start=True, stop=True)
                    # S^T[g, n] = (batchloc[n] == g)   [one op, 512 wide]
                    StT = pool.tile([P, ST * P], bf16, tag="StT")
                    nc.vector.tensor_scalar(out=StT[:, :w], in0=bc_ps[:, :w],
                                            scalar1=iota_col[:], scalar2=None,
                                            op0=EQ)
                    # S[n, g] per 128-node subtile
                    S_st = pool.tile([P, ST * P], bf16, tag="S")
                    for c in range(nsub):
                        csl = slice(c * P, (c + 1) * P)
                        nc.vector.tensor_scalar(
                            out=S_st[:, csl], in0=iota_row[:],
                            scalar1=blc[:, st * ST + c:st * ST + c + 1],
                            scalar2=None, op0=EQ)
                    # pre^T[d, n] = W2 @ x^T + q1g^T-expand   (+b12 in ACT)
                    pre_ps = psPre.tile([P, ST * P], f32, tag="pre", space="PSUM")
                    nc.tensor.matmul(out=pre_ps[:, :w], lhsT=w2t[:],
                                     rhs=xtp[:, ssl], start=True, stop=False)
                    nc.tensor.matmul(out=pre_ps[:, :w], lhsT=q1g[:],
                                     rhs=StT[:, :w], start=False, stop=True)
                    sigT = pool.tile([P, ST * P], bf16, tag="sigT")
                    nc.scalar.activation(out=sigT[:, :w], in_=pre_ps[:, :w],
                                         func=Sig, bias=b12c[:])
                    # alpha[n] = sig @ q_w.T (+ q_b in copy)
                    al_ps = psAl.tile([P, ST], f32, tag="al", space="PSUM")
                    for c in range(nsub):
                        csl = slice(c * P, (c + 1) * P)
                        nc.tensor.matmul(out=al_ps[:, c:c + 1],
                                         lhsT=sigT[:, csl], rhs=qwt[:],
                                         start=True, stop=True)
                    al = pool.tile([P, ST], f32, tag="al")
                    nc.vector.tensor_scalar_add(out=al[:, :nsub],
                                                in0=al_ps[:, :nsub],
                                                scalar1=qbc[:])
                    # xa = alpha * x ; s_g^T[d, g] += xa^T-reduce via S
                    xa = pool.tile([P, ST * P], bf16, tag="xa")
                    for c in range(nsub):
                        csl = slice(c * P, (c + 1) * P)
                        nc.vector.tensor_scalar_mul(
                            out=xa[:, csl],
                            in0=xpk[:, st * ST * P + c * P:st * ST * P + (c + 1) * P],
                            scalar1=al[:, c:c + 1])
                        nc.tensor.matmul(out=sg_ps, lhsT=xa[:, csl],
                                         rhs=S_st[:, csl],
                                         start=(mm_i == 0), stop=(mm_i == n_mm - 1))
                        mm_i += 1

                sg_sb = blkp.tile([P, P], f32, tag="sgsb")
                nc.vector.tensor_copy(out=sg_sb[:], in_=sg_ps)
                # s_h^T[d, g] = W3a @ v_n^T + W3b @ s_g^T  (+W3_b in copy)
                sh_ps = psS_t[:, 0:P]
                nc.tensor.matmul(out=sh_ps, lhsT=w3at[:], rhs=vnt[:, gsl],
                                 start=True, stop=False)
                nc.tensor.matmul(out=sh_ps, lhsT=w3bt[:], rhs=sg_sb[:],
                                 start=False, stop=True)
                nc.vector.tensor_scalar_add(out=shT[:, gsl], in0=sh_ps,
                                            scalar1=w3bc[:])

            if do2:
                # --------- phase 2 for this block: s_h @ item_weight^T -----
                for g in range(ngrp):
                    gvsl = slice(g * Wg, (g + 1) * Wg)
                    if g < len(itw_pre):
                        itw = itw_pre[g]
                    else:
                        itw = p2l.tile([D, Wg], bf16, tag="itw")
                        nc.sync.dma_start(out=itw[:], in_=itwt_ext[:, gvsl])
                    sc = p2o.tile([P, Wg], f32, tag="scsb")
                    for s in range(VG):
                        sc_ps = ps2.tile([P, VT], f32, tag="sc", space="PSUM")
                        nc.tensor.matmul(out=sc_ps[:],
                                         lhsT=shT[:, gsl],
                                         rhs=itw[:, s * VT:(s + 1) * VT],
                                         start=True, stop=True)
                        if s % 2 == 0:
                            nc.vector.tensor_copy(out=sc[:, s * VT:(s + 1) * VT],
                                                  in_=sc_ps[:])
                        else:
                            nc.scalar.copy(out=sc[:, s * VT:(s + 1) * VT],
                                           in_=sc_ps[:])
                    nc.sync.dma_start(out=y_ext[blk * P:(blk + 1) * P, gvsl],
                                      in_=sc[:])


def prep_inputs(session_embedding, item_weight, W1_w, W1_b, W2_w, W2_b,
                q_w, q_b, W3_w, W3_b, batch, num_graphs):
    """Host-side sharding/layout. Returns (in_maps, ntpb, vpad, V, cc)."""
    x = np.ascontiguousarray(np.asarray(session_embedding, dtype=np.float32))
    itw = np.asarray(item_weight, dtype=np.float32)
    batch = np.asarray(batch).astype(np.int64)
    B = int(num_graphs)
    N, d = x.shape
    V = itw.shape[0]
    assert d == D and B == NCORES * BC, (d, B)

    counts = np.bincount(batch, minlength=B)
    assert counts.min() >= 1, "every graph must be non-empty"
    starts = np.zeros(B + 1, np.int64)
    np.cumsum(counts, out=starts[1:])
    assert starts[-1] == N
    last_idx = starts[1:] - 1
    v_n = x[last_idx]                                   # [B, D]

    blk_cnt = starts[P::P] - starts[:-P:P].reshape(-1)  # [B//P]
    ntpb = int(-(-blk_cnt.max() // P))                  # ceil
    npb = ntpb * P

    vpad = -(-V // (VT * VG)) * (VT * VG)
    itwT = np.zeros((D, vpad), BF16)
    itwT[:, :V] = itw.T.astype(BF16)

    w1t = np.ascontiguousarray(np.asarray(W1_w, np.float32).T)
    w2t = np.ascontiguousarray(np.asarray(W2_w, np.float32).T.astype(BF16))
    W3 = np.asarray(W3_w, np.float32)
    w3at = np.ascontiguousarray(W3[:, :D].T)
    w3bt = np.ascontiguousarray(W3[:, D:].T)
    b12c = (np.asarray(W1_b, np.float32) + np.asarray(W2_b, np.float32)
            ).reshape(P, 1).copy()
    w3bc = np.asarray(W3_b, np.float32).reshape(P, 1).copy()
    qwt = np.ascontiguousarray(
        np.asarray(q_w, np.float32).reshape(1, D).T.astype(BF16))
    sel4 = np.zeros((NBLK, NBLK * P), BF16)
    for k in range(NBLK):
        sel4[k, k * P:(k + 1) * P] = 1
    qbc = np.full((P, 1), np.float32(np.asarray(q_b).reshape(())), np.float32)

    in_maps = []
    for c in range(NCORES):
        xpad = np.zeros((NBLK, npb, D), np.float32)
        bl = np.zeros((NBLK, P, ntpb), np.float32)
        blr = np.zeros((NBLK, npb), BF16)
        for b in range(NBLK):
            glo = c * BC + b * P
            s, e = int(starts[glo]), int(starts[glo + P])
            cnt = e - s
            assert cnt <= npb
            xpad[b, :cnt] = x[s:e]
            locp = np.zeros(npb, np.float32)
            locp[:cnt] = (batch[s:e] - glo).astype(np.float32)
            bl[b] = locp.reshape(ntpb, P).T
            blr[b] = locp.astype(BF16)
        # packed node-row: xpk[:, blk*npb + t*128 + j][i] = x_pad[blk, t*128+i, j]
        xpk = np.ascontiguousarray(
            xpad.reshape(NBLK, ntpb, P, D).transpose(2, 0, 1, 3)
            .reshape(P, NBLK * npb).astype(BF16))
        # feature-row transposed: xtp[:, blk*npb + n] = x_pad[blk, n, :]
        xtp = np.ascontiguousarray(
            xpad.transpose(2, 0, 1).reshape(P, NBLK * npb).astype(BF16))
        vnt = np.ascontiguousarray(v_n[c * BC:(c + 1) * BC].T)
        im = dict(
            xpk=xpk, xtp=xtp, blc=np.ascontiguousarray(bl),
            blr=np.ascontiguousarray(blr), vnt=vnt,
            w1t=w1t, w2t=w2t, w3at=w3at, w3bt=w3bt,
            b12c=b12c, w3bc=w3bc, qwt=qwt, qbc=qbc, sel4=sel4, itwt=itwT)
        in_maps.append(im)
    return in_maps, ntpb, vpad, V, False


_NC_CACHE = {}


def get_nc(ntpb, vpad, repeat=1, phase="both", cc=False):
    key = (ntpb, vpad, repeat, phase)
    if key not in _NC_CACHE:
        _NC_CACHE[key] = build_nc(ntpb, vpad, repeat, phase)
    return _NC_CACHE[key]


def kernel(**inputs):
    from concourse.bass_utils import run_bass_kernel_spmd

    in_maps, ntpb, vpad, V, cc = prep_inputs(**inputs)
    nc = get_nc(ntpb, vpad)
    res = run_bass_kernel_spmd(nc, in_maps, core_ids=list(range(NCORES)))
    B = int(inputs["num_graphs"])
    y = np.empty((B, V), np.float32)
    for c in range(NCORES):
        y[c * BC:(c + 1) * BC] = res.results[c]["y"][:, :V]
    return y


# revision 18
# speedup vs baseline: 1.8374x; 1.0525x over previous
"""Trainium2 Bass kernel for nn_Embedding2Score (segment_reduce).

Strategy (data-parallel over sessions, per sharding hint):
  - 4096 graphs -> 8 cores x 512 graphs (4 blocks of 128 graphs each).
    Each core owns whole contiguous segments (batch is sorted by graph).
  - Nodes are processed in 512-node supertiles (4 x 128-node subtiles).
    Segment broadcast (v_n -> nodes) and segment sum (alpha*x -> s_g) are
    one-hot matmuls on PE; one-hot blocks are built with DVE is_equal.
    The graph-major one-hot S^T needs batchloc replicated down the 128
    partitions: done with a K=1 ones-matmul into PSUM (no DMA broadcast).
  - bf16 data path: x (both layouts), item_weight, W2, q_w and all one-hot
    matrices are bf16 (PE runs 1 cycle/row vs 4 for fp32, DMA bytes halve).
    Accumulation stays fp32 in PSUM; biases/alpha/v_n/W1/W3 stay fp32.
    rel-err budget is 2e-2; bf16 keeps it ~2e-3.
  - Final scoring: s_h^T [128d, 512g] per core vs item_weight^T tiles,
    grouped 4 vocab-tiles per DMA so all phase-2 DMAs are ~1 MB.
    Output rows [512, V] per core = row-slice of [4096, 50000].
"""

import sys

if "/opt/trn_rl_repo" not in sys.path:
    sys.path.insert(0, "/opt/trn_rl_repo")

import numpy as np
import ml_dtypes

BF16 = ml_dtypes.bfloat16
P = 128          # partitions / tile edge
D = 128          # hidden size
NCORES = 8
NBLK = 4         # graph blocks per core, 128 graphs each
BC = NBLK * P    # graphs per core = 512
VT = 500         # vocab tile (psum bank holds 512 fp32)
VG = 4           # vocab tiles per DMA group
ST = 4           # 128-node subtiles per supertile
NPF = 25         # vocab groups prefetched (all SBUF-resident)


def build_nc(ntpb, vpad, repeat=1, phase="both"):
    """Build the per-core Bass program. ntpb = node tiles per graph-block,
    vpad = padded vocab size (multiple of VT*VG). repeat>1 wraps the body
    in a hardware loop (timing probes). phase: 'both' | 'p1' | 'p2'."""
    import contextlib
    import concourse.bacc as bacc
    import concourse.mybir as mybir
    from concourse.tile import TileContext

    f32 = mybir.dt.float32
    bf16 = mybir.dt.bfloat16
    npb = ntpb * P
    nc = bacc.Bacc()

    xpk_ext = nc.declare_dram_parameter("xpk", [P, NBLK * npb], bf16, isOutput=False)
    xtp_ext = nc.declare_dram_parameter("xtp", [P, NBLK * npb], bf16, isOutput=False)
    blc_ext = nc.declare_dram_parameter("blc", [NBLK, P, ntpb], f32, isOutput=False)
    blr_ext = nc.declare_dram_parameter("blr", [NBLK, npb], bf16, isOutput=False)
    vnt_ext = nc.declare_dram_parameter("vnt", [D, BC], f32, isOutput=False)
    w1t_ext = nc.declare_dram_parameter("w1t", [D, D], f32, isOutput=False)
    w2t_ext = nc.declare_dram_parameter("w2t", [D, D], bf16, isOutput=False)
    w3at_ext = nc.declare_dram_parameter("w3at", [D, D], f32, isOutput=False)
    w3bt_ext = nc.declare_dram_parameter("w3bt", [D, D], f32, isOutput=False)
    b12c_ext = nc.declare_dram_parameter("b12c", [P, 1], f32, isOutput=False)
    w3bc_ext = nc.declare_dram_parameter("w3bc", [P, 1], f32, isOutput=False)
    qwt_ext = nc.declare_dram_parameter("qwt", [D, 1], bf16, isOutput=False)
    sel4_ext = nc.declare_dram_parameter("sel4", [NBLK, NBLK * P], bf16,
                                         isOutput=False)
    qbc_ext = nc.declare_dram_parameter("qbc", [P, 1], f32, isOutput=False)
    itwt_ext = nc.declare_dram_parameter("itwt", [D, vpad], bf16, isOutput=False)
    y_ext = nc.declare_dram_parameter("y", [BC, vpad], f32, isOutput=True)

    with TileContext(nc) as tc:
        with tc.tile_pool(name="const", bufs=1) as cp:
            iota_i = cp.tile([P, P], mybir.dt.int32, tag="iotai")
            nc.gpsimd.iota(iota_i[:], pattern=[[1, P]], base=0, channel_multiplier=0)
            iota_f = cp.tile([P, P], f32, tag="iotaf")
            nc.vector.tensor_copy(out=iota_f[:], in_=iota_i[:])
            iota_row = cp.tile([P, P], bf16, tag="iotarow")
            nc.vector.tensor_copy(out=iota_row[:], in_=iota_f[:])
            iota_ci = cp.tile([P, 1], mybir.dt.int32, tag="iotaci")
            nc.gpsimd.iota(iota_ci[:], pattern=[[0, 1]], base=0, channel_multiplier=1)
            iota_col = cp.tile([P, 1], f32, tag="iotacol")
            nc.vector.tensor_copy(out=iota_col[:], in_=iota_ci[:])
            ones_row = cp.tile([1, P], bf16, tag="onesrow")
            nc.vector.memset(ones_row[:], 1.0)

            def load(name, ext, shape, dt):
                t = cp.tile(shape, dt, tag=name)
                nc.sync.dma_start(out=t[:], in_=ext[:])
                return t

            w1t = load("w1t", w1t_ext, [D, D], f32)
            w2t = load("w2t", w2t_ext, [D, D], bf16)
            w3at = load("w3at", w3at_ext, [D, D], f32)
            w3bt = load("w3bt", w3bt_ext, [D, D], f32)
            b12c = load("b12c", b12c_ext, [P, 1], f32)
            w3bc = load("w3bc", w3bc_ext, [P, 1], f32)
            qwt = load("qwt", qwt_ext, [D, 1], bf16)
            qbc = load("qbc", qbc_ext, [P, 1], f32)
            vnt = load("vnt", vnt_ext, [D, BC], f32)

            shT = cp.tile([D, BC], bf16, tag="shT")  # s_h^T, filled per block
            if phase == "p2":
                nc.vector.memset(shT[:], 0.01)
            # per-block batchloc tables are iteration-invariant: load once
            blc_sb = []
            for blk in range(NBLK):
                t = cp.tile([P, ntpb], f32, tag=f"blc{blk}")
                nc.sync.dma_start(out=t[:], in_=blc_ext[blk])
                blc_sb.append(t)
            blr4 = cp.tile([NBLK, npb], bf16, tag="blr4")
            nc.sync.dma_start(out=blr4[:], in_=blr_ext[:])
            sel4 = cp.tile([NBLK, NBLK * P], bf16, tag="sel4")
            nc.sync.dma_start(out=sel4[:], in_=sel4_ext[:])
            itw_pre = []
            W = VG * VT
            for g in range(min(NPF, vpad // W)):
                t = cp.tile([D, W], bf16, tag=f"itwpre{g}")
                nc.sync.dma_start(out=t[:], in_=itwt_ext[:, g * W:(g + 1) * W])
                itw_pre.append(t)

            rep_ctx = tc.For_i(0, repeat, 1) if repeat > 1 else contextlib.nullcontext()
            with rep_ctx:
                _build_body(nc, tc, mybir, ntpb, vpad,
                            xpk_ext, xtp_ext, blc_sb, blr4, sel4, itwt_ext, y_ext,
                            iota_row, iota_col, ones_row,
                            w1t, w2t, w3at, w3bt, b12c, w3bc, qwt, qbc, vnt, shT,
                            phase, itw_pre)

    nc.compile()
    return nc


def _build_body(nc, tc, mybir, ntpb, vpad,
                xpk_ext, xtp_ext, blc_sb, blr4, sel4, itwt_ext, y_ext,
                iota_row, iota_col, ones_row,
                w1t, w2t, w3at, w3bt, b12c, w3bc, qwt, qbc, vnt, shT,
                phase="both", itw_pre=None):
    f32 = mybir.dt.float32
    bf16 = mybir.dt.bfloat16
    npb = ntpb * P
    nst = -(-ntpb // ST)          # supertiles per block
    Sig = mybir.ActivationFunctionType.Sigmoid
    EQ = mybir.AluOpType.is_equal
    Wg = VG * VT
    ngrp = vpad // Wg
    do1 = phase in ("both", "p1")
    do2 = phase in ("both", "p2")

    # phase 1 (attention + segment reduce) and phase 2 (vocab scoring) are
    # software-pipelined: block b+1's phase-1 work is emitted in per-supertile
    # chunks between block b's phase-2 vocab groups, so the in-order engines
    # never expose phase-1 latency at block boundaries and the y-store stream
    # stays saturated.
    with tc.tile_pool(name="p1big", bufs=2) as pb, \
         tc.tile_pool(name="p1", bufs=2) as pool, \
         tc.tile_pool(name="blkp", bufs=2) as blkp, \
         tc.tile_pool(name="p2o", bufs=6) as p2o, \
         tc.tile_pool(name="psPre", bufs=2, space="PSUM") as psPre, \
         tc.tile_pool(name="psBc", bufs=1, space="PSUM") as psBc, \
         tc.tile_pool(name="psS", bufs=1, space="PSUM") as psS, \
         tc.tile_pool(name="psAl", bufs=1, space="PSUM") as psAl, \
         tc.tile_pool(name="ps2", bufs=3, space="PSUM") as ps2:

        def p1_segments(blk):
            """Emit phase 1 for `blk` as a list of closures (prologue,
            one per supertile, epilogue) sharing state via `sv`."""
            gsl = slice(blk * P, (blk + 1) * P)
            nsl = slice(blk * npb, (blk + 1) * npb)
            sv = {}
            n_mm = sum(min(ST, ntpb - ST * s) for s in range(nst))

            def prologue():
                sv["psS_t"] = psS.tile([P, 4 * P], f32, tag="psS", name="psS_t",
                                       space="PSUM")
                xpk = pb.tile([P, npb], bf16, tag="xpk")
                nc.sync.dma_start(out=xpk[:], in_=xpk_ext[:, nsl])
                xtp = pb.tile([P, npb], bf16, tag="xtp")
                nc.sync.dma_start(out=xtp[:], in_=xtp_ext[:, nsl])
                sv["xpk"], sv["xtp"] = xpk, xtp
                # q1g[g, d] = (v_n_blk @ W1_w.T)[g, d] (biases folded later)
                q1g_ps = sv["psS_t"][:, 0:P]
                nc.tensor.matmul(out=q1g_ps, lhsT=vnt[:, gsl], rhs=w1t[:],
                                 start=True, stop=True)
                q1g = blkp.tile([P, P], bf16, tag="q1g")
                nc.vector.tensor_copy(out=q1g[:], in_=q1g_ps)
                sv["q1g"] = q1g
                sv["mm_i"] = 0

            def supertile(st):
                blc = blc_sb[blk]
                q1g, xpk, xtp = sv["q1g"], sv["xpk"], sv["xtp"]
                sg_ps = sv["psS_t"][:, P:2 * P]
                nsub = min(ST, ntpb - ST * st)
                w = nsub * P
                ssl = slice(st * ST * P, st * ST * P + w)  # cols in block
                # batchloc replicated down partitions via K=4 row-select
                bc_ps = psBc.tile([P, ST * P], f32, tag="bc", space="PSUM")
                nc.tensor.matmul(out=bc_ps[:, :w],
                                 lhsT=sel4[:, blk * P:(blk + 1) * P],
                                 rhs=blr4[:, ssl], start=True, stop=True)
                # S^T[g, n] = (batchloc[n] == g)   [one op, 512 wide]
                StT = pool.tile([P, ST * P], bf16, tag="StT")
                nc.vector.tensor_scalar(out=StT[:, :w], in0=bc_ps[:, :w],
                                        scalar1=iota_col[:], scalar2=None,
                                        op0=EQ)
                # S[n, g] per 128-node subtile
                S_st = pool.tile([P, ST * P], bf16, tag="S")
                for c in range(nsub):
                    csl = slice(c * P, (c + 1) * P)
                    nc.vector.tensor_scalar(
                        out=S_st[:, csl], in0=iota_row[:],
                        scalar1=blc[:, st * ST + c:st * ST + c + 1],
                        scalar2=None, op0=EQ)
                # pre^T[d, n] = W2 @ x^T + q1g^T-expand   (+b12 in ACT)
                pre_ps = psPre.tile([P, ST * P], f32, tag="pre", space="PSUM")
                nc.tensor.matmul(out=pre_ps[:, :w], lhsT=w2t[:],
                                 rhs=xtp[:, ssl], start=True, stop=False)
                nc.tensor.matmul(out=pre_ps[:, :w], lhsT=q1g[:],
                                 rhs=StT[:, :w], start=False, stop=True)
                sigT = pool.tile([P, ST * P], bf16, tag="sigT")
                nc.scalar.activation(out=sigT[:, :w], in_=pre_ps[:, :w],
                                     func=Sig, bias=b12c[:])
                # alpha[n] = sig @ q_w.T (+ q_b in copy)
                al_ps = psAl.tile([P, ST], f32, tag="al", space="PSUM")
                for c in range(nsub):
                    csl = slice(c * P, (c + 1) * P)
                    nc.tensor.matmul(out=al_ps[:, c:c + 1],
                                     lhsT=sigT[:, csl], rhs=qwt[:],
                                     start=True, stop=True)
                al = pool.tile([P, ST], f32, tag="al")
                nc.vector.tensor_scalar_add(out=al[:, :nsub],
                                            in0=al_ps[:, :nsub],
                                            scalar1=qbc[:])
                # xa = alpha * x ; s_g^T[d, g] += xa^T-reduce via S
                xa = pool.tile([P, ST * P], bf16, tag="xa")
                for c in range(nsub):
                    csl = slice(c * P, (c + 1) * P)
                    nc.vector.tensor_scalar_mul(
                        out=xa[:, csl],
                        in0=xpk[:, st * ST * P + c * P:st * ST * P + (c + 1) * P],
                        scalar1=al[:, c:c + 1])
                    nc.tensor.matmul(out=sg_ps, lhsT=xa[:, csl],
                                     rhs=S_st[:, csl],
                                     start=(sv["mm_i"] == 0),
                                     stop=(sv["mm_i"] == n_mm - 1))
                    sv["mm_i"] += 1

            def epilogue():
                sg_ps = sv["psS_t"][:, P:2 * P]
                sg_sb = blkp.tile([P, P], f32, tag="sgsb")
                nc.vector.tensor_copy(out=sg_sb[:], in_=sg_ps)
                # s_h^T[d, g] = W3a @ v_n^T + W3b @ s_g^T  (+W3_b in copy)
                sh_ps = sv["psS_t"][:, 0:P]
                nc.tensor.matmul(out=sh_ps, lhsT=w3at[:], rhs=vnt[:, gsl],
                                 start=True, stop=False)
                nc.tensor.matmul(out=sh_ps, lhsT=w3bt[:], rhs=sg_sb[:],
                                 start=False, stop=True)
                nc.vector.tensor_scalar_add(out=shT[:, gsl], in0=sh_ps,
                                            scalar1=w3bc[:])

            return ([prologue]
                    + [lambda st=st: supertile(st) for st in range(nst)]
                    + [epilogue])

        def p2_group(blk, g):
            gsl = slice(blk * P, (blk + 1) * P)
            gvsl = slice(g * Wg, (g + 1) * Wg)
            itw = itw_pre[g]
            sc = p2o.tile([P, Wg], f32, tag="scsb")
            for s in range(VG):
                sc_ps = ps2.tile([P, VT], f32, tag="sc", space="PSUM")
                nc.tensor.matmul(out=sc_ps[:],
                                 lhsT=shT[:, gsl],
                                 rhs=itw[:, s * VT:(s + 1) * VT],
                                 start=True, stop=True)
                if s % 2 == 0:
                    nc.vector.tensor_copy(out=sc[:, s * VT:(s + 1) * VT],
                                          in_=sc_ps[:])
                else:
                    nc.scalar.copy(out=sc[:, s * VT:(s + 1) * VT],
                                   in_=sc_ps[:])
            nc.sync.dma_start(out=y_ext[blk * P:(blk + 1) * P, gvsl],
                              in_=sc[:])

        if not do2:
            for blk in range(NBLK):
                for seg in p1_segments(blk):
                    seg()
            return
        if do1:
            for seg in p1_segments(0):
                seg()
        for blk in range(NBLK):
            # interleave next block's phase-1 chunks between vocab groups,
            # starting a couple of groups in so the store queue fills first
            nxt = list(p1_segments(blk + 1)) if (do1 and blk + 1 < NBLK) else []
            for g in range(ngrp):
                p2_group(blk, g)
                if nxt and g >= 1:
                    nxt.pop(0)()
                    if len(nxt) > ngrp - g - 2:   # don't leave a tail
                        nxt.pop(0)()
            while nxt:
                nxt.pop(0)()


def prep_inputs(session_embedding, item_weight, W1_w, W1_b, W2_w, W2_b,
                q_w, q_b, W3_w, W3_b, batch, num_graphs):
    """Host-side sharding/layout. Returns (in_maps, ntpb, vpad, V, cc)."""
    x = np.ascontiguousarray(np.asarray(session_embedding, dtype=np.float32))
    itw = np.asarray(item_weight, dtype=np.float32)
    batch = np.asarray(batch).astype(np.int64)
    B = int(num_graphs)
    N, d = x.shape
    V = itw.shape[0]
    assert d == D and B == NCORES * BC, (d, B)

    counts = np.bincount(batch, minlength=B)
    assert counts.min() >= 1, "every graph must be non-empty"
    starts = np.zeros(B + 1, np.int64)
    np.cumsum(counts, out=starts[1:])
    assert starts[-1] == N
    last_idx = starts[1:] - 1
    v_n = x[last_idx]                                   # [B, D]

    blk_cnt = starts[P::P] - starts[:-P:P].reshape(-1)  # [B//P]
    ntpb = int(-(-blk_cnt.max() // P))                  # ceil
    npb = ntpb * P

    vpad = -(-V // (VT * VG)) * (VT * VG)
    itwT = np.zeros((D, vpad), BF16)
    itwT[:, :V] = itw.T.astype(BF16)

    w1t = np.ascontiguousarray(np.asarray(W1_w, np.float32).T)
    w2t = np.ascontiguousarray(np.asarray(W2_w, np.float32).T.astype(BF16))
    W3 = np.asarray(W3_w, np.float32)
    w3at = np.ascontiguousarray(W3[:, :D].T)
    w3bt = np.ascontiguousarray(W3[:, D:].T)
    b12c = (np.asarray(W1_b, np.float32) + np.asarray(W2_b, np.float32)
            ).reshape(P, 1).copy()
    w3bc = np.asarray(W3_b, np.float32).reshape(P, 1).copy()
    qwt = np.ascontiguousarray(
        np.asarray(q_w, np.float32).reshape(1, D).T.astype(BF16))
    sel4 = np.zeros((NBLK, NBLK * P), BF16)
    for k in range(NBLK):
        sel4[k, k * P:(k + 1) * P] = 1
    qbc = np.full((P, 1), np.float32(np.asarray(q_b).reshape(())), np.float32)

    in_maps = []
    for c in range(NCORES):
        xpad = np.zeros((NBLK, npb, D), np.float32)
        bl = np.zeros((NBLK, P, ntpb), np.float32)
        blr = np.zeros((NBLK, npb), BF16)
        for b in range(NBLK):
            glo = c * BC + b * P
            s, e = int(starts[glo]), int(starts[glo + P])
            cnt = e - s
            assert cnt <= npb
            xpad[b, :cnt] = x[s:e]
            locp = np.zeros(npb, np.float32)
            locp[:cnt] = (batch[s:e] - glo).astype(np.float32)
            bl[b] = locp.reshape(ntpb, P).T
            blr[b] = locp.astype(BF16)
        # packed node-row: xpk[:, blk*npb + t*128 + j][i] = x_pad[blk, t*128+i, j]
        xpk = np.ascontiguousarray(
            xpad.reshape(NBLK, ntpb, P, D).transpose(2, 0, 1, 3)
            .reshape(P, NBLK * npb).astype(BF16))
        # feature-row transposed: xtp[:, blk*npb + n] = x_pad[blk, n, :]
        xtp = np.ascontiguousarray(
            xpad.transpose(2, 0, 1).reshape(P, NBLK * npb).astype(BF16))
        vnt = np.ascontiguousarray(v_n[c * BC:(c + 1) * BC].T)
        im = dict(
            xpk=xpk, xtp=xtp, blc=np.ascontiguousarray(bl),
            blr=np.ascontiguousarray(blr), vnt=vnt,
            w1t=w1t, w2t=w2t, w3at=w3at, w3bt=w3bt,
            b12c=b12c, w3bc=w3bc, qwt=qwt, qbc=qbc, sel4=sel4, itwt=itwT)
        in_maps.append(im)
    return in_maps, ntpb, vpad, V, False


_NC_CACHE = {}


def get_nc(ntpb, vpad, repeat=1, phase="both", cc=False):
    key = (ntpb, vpad, repeat, phase)
    if key not in _NC_CACHE:
        _NC_CACHE[key] = build_nc(ntpb, vpad, repeat, phase)
    return _NC_CACHE[key]


def kernel(**inputs):
    from concourse.bass_utils import run_bass_kernel_spmd

    in_maps, ntpb, vpad, V, cc = prep_inputs(**inputs)
    nc = get_nc(ntpb, vpad)
    res = run_bass_kernel_spmd(nc, in_maps, core_ids=list(range(NCORES)))
    B = int(inputs["num_graphs"])
    y = np.empty((B, V), np.float32)
    for c in range(NCORES):
        y[c * BC:(c + 1) * BC] = res.results[c]["y"][:, :V]
    return y


# revision 19
# speedup vs baseline: 2.6818x; 1.4595x over previous
"""Trainium2 Bass kernel for nn_Embedding2Score (segment_reduce).

Strategy (data-parallel over sessions, per sharding hint):
  - 4096 graphs -> 8 cores x 512 graphs (4 blocks of 128 graphs each).
    Each core owns whole contiguous segments (batch is sorted by graph).
  - Nodes are processed in 512-node supertiles (4 x 128-node subtiles).
    Segment broadcast (v_n -> nodes) and segment sum (alpha*x -> s_g) are
    one-hot matmuls on PE; one-hot blocks are built with DVE is_equal.
    The graph-major one-hot S^T needs batchloc replicated down the 128
    partitions: done with a K=1 ones-matmul into PSUM (no DMA broadcast).
  - bf16 data path: x (both layouts), item_weight, W2, q_w and all one-hot
    matrices are bf16 (PE runs 1 cycle/row vs 4 for fp32, DMA bytes halve).
    Accumulation stays fp32 in PSUM; biases/alpha/v_n/W1/W3 stay fp32.
    rel-err budget is 2e-2; bf16 keeps it ~2e-3.
  - Final scoring: s_h^T [128d, 512g] per core vs item_weight^T tiles,
    grouped 4 vocab-tiles per DMA so all phase-2 DMAs are ~1 MB.
    Output rows [512, V] per core = row-slice of [4096, 50000].
"""

import sys

if "/opt/trn_rl_repo" not in sys.path:
    sys.path.insert(0, "/opt/trn_rl_repo")

import numpy as np
import ml_dtypes

BF16 = ml_dtypes.bfloat16
P = 128          # partitions / tile edge
D = 128          # hidden size
NCORES = 8
NBLK = 4         # graph blocks per core, 128 graphs each
BC = NBLK * P    # graphs per core = 512
VT = 500         # vocab tile (psum bank holds 512 fp32)
VG = 4           # vocab tiles per DMA group
ST = 4           # 128-node subtiles per supertile
NPF = 25         # vocab groups prefetched (all SBUF-resident)


def build_nc(ntpb, vpad, repeat=1, phase="both"):
    """Build the per-core Bass program. ntpb = node tiles per graph-block,
    vpad = padded vocab size (multiple of VT*VG). repeat>1 wraps the body
    in a hardware loop (timing probes). phase: 'both' | 'p1' | 'p2'."""
    import contextlib
    import concourse.bacc as bacc
    import concourse.mybir as mybir
    from concourse.tile import TileContext

    f32 = mybir.dt.float32
    bf16 = mybir.dt.bfloat16
    npb = ntpb * P
    nc = bacc.Bacc()

    xpk_ext = nc.declare_dram_parameter("xpk", [P, NBLK * npb], bf16, isOutput=False)
    xtp_ext = nc.declare_dram_parameter("xtp", [P, NBLK * npb], bf16, isOutput=False)
    blc_ext = nc.declare_dram_parameter("blc", [NBLK, P, ntpb], f32, isOutput=False)
    blr_ext = nc.declare_dram_parameter("blr", [NBLK, npb], bf16, isOutput=False)
    vnt_ext = nc.declare_dram_parameter("vnt", [D, BC], f32, isOutput=False)
    w1t_ext = nc.declare_dram_parameter("w1t", [D, D], f32, isOutput=False)
    w2t_ext = nc.declare_dram_parameter("w2t", [D, D], bf16, isOutput=False)
    w3at_ext = nc.declare_dram_parameter("w3at", [D, D], f32, isOutput=False)
    w3bt_ext = nc.declare_dram_parameter("w3bt", [D, D], f32, isOutput=False)
    b12c_ext = nc.declare_dram_parameter("b12c", [P, 1], f32, isOutput=False)
    w3bc_ext = nc.declare_dram_parameter("w3bc", [P, 1], f32, isOutput=False)
    qwt_ext = nc.declare_dram_parameter("qwt", [D, 1], bf16, isOutput=False)
    sel4_ext = nc.declare_dram_parameter("sel4", [NBLK, NBLK * P], bf16,
                                         isOutput=False)
    qbc_ext = nc.declare_dram_parameter("qbc", [P, 1], f32, isOutput=False)
    itwt_ext = nc.declare_dram_parameter("itwt", [D, vpad], bf16, isOutput=False)
    y_ext = nc.declare_dram_parameter("y", [BC, vpad], bf16, isOutput=True)

    with TileContext(nc) as tc:
        with tc.tile_pool(name="const", bufs=1) as cp:
            iota_i = cp.tile([P, P], mybir.dt.int32, tag="iotai")
            nc.gpsimd.iota(iota_i[:], pattern=[[1, P]], base=0, channel_multiplier=0)
            iota_f = cp.tile([P, P], f32, tag="iotaf")
            nc.vector.tensor_copy(out=iota_f[:], in_=iota_i[:])
            iota_row = cp.tile([P, P], bf16, tag="iotarow")
            nc.vector.tensor_copy(out=iota_row[:], in_=iota_f[:])
            iota_ci = cp.tile([P, 1], mybir.dt.int32, tag="iotaci")
            nc.gpsimd.iota(iota_ci[:], pattern=[[0, 1]], base=0, channel_multiplier=1)
            iota_col = cp.tile([P, 1], f32, tag="iotacol")
            nc.vector.tensor_copy(out=iota_col[:], in_=iota_ci[:])
            ones_row = cp.tile([1, P], bf16, tag="onesrow")
            nc.vector.memset(ones_row[:], 1.0)

            def load(name, ext, shape, dt):
                t = cp.tile(shape, dt, tag=name)
                nc.sync.dma_start(out=t[:], in_=ext[:])
                return t

            w1t = load("w1t", w1t_ext, [D, D], f32)
            w2t = load("w2t", w2t_ext, [D, D], bf16)
            w3at = load("w3at", w3at_ext, [D, D], f32)
            w3bt = load("w3bt", w3bt_ext, [D, D], f32)
            b12c = load("b12c", b12c_ext, [P, 1], f32)
            w3bc = load("w3bc", w3bc_ext, [P, 1], f32)
            qwt = load("qwt", qwt_ext, [D, 1], bf16)
            qbc = load("qbc", qbc_ext, [P, 1], f32)
            vnt = load("vnt", vnt_ext, [D, BC], f32)

            shT = cp.tile([D, BC], bf16, tag="shT")  # s_h^T, filled per block
            if phase == "p2":
                nc.vector.memset(shT[:], 0.01)
            # per-block batchloc tables are iteration-invariant: load once
            blc_sb = []
            for blk in range(NBLK):
                t = cp.tile([P, ntpb], f32, tag=f"blc{blk}")
                nc.sync.dma_start(out=t[:], in_=blc_ext[blk])
                blc_sb.append(t)
            blr4 = cp.tile([NBLK, npb], bf16, tag="blr4")
            nc.sync.dma_start(out=blr4[:], in_=blr_ext[:])
            sel4 = cp.tile([NBLK, NBLK * P], bf16, tag="sel4")
            nc.sync.dma_start(out=sel4[:], in_=sel4_ext[:])
            itw_pre = []
            W = VG * VT
            for g in range(min(NPF, vpad // W)):
                t = cp.tile([D, W], bf16, tag=f"itwpre{g}")
                nc.sync.dma_start(out=t[:], in_=itwt_ext[:, g * W:(g + 1) * W])
                itw_pre.append(t)

            rep_ctx = tc.For_i(0, repeat, 1) if repeat > 1 else contextlib.nullcontext()
            with rep_ctx:
                _build_body(nc, tc, mybir, ntpb, vpad,
                            xpk_ext, xtp_ext, blc_sb, blr4, sel4, itwt_ext, y_ext,
                            iota_row, iota_col, ones_row,
                            w1t, w2t, w3at, w3bt, b12c, w3bc, qwt, qbc, vnt, shT,
                            phase, itw_pre)

    nc.compile()
    return nc


def _build_body(nc, tc, mybir, ntpb, vpad,
                xpk_ext, xtp_ext, blc_sb, blr4, sel4, itwt_ext, y_ext,
                iota_row, iota_col, ones_row,
                w1t, w2t, w3at, w3bt, b12c, w3bc, qwt, qbc, vnt, shT,
                phase="both", itw_pre=None):
    f32 = mybir.dt.float32
    bf16 = mybir.dt.bfloat16
    npb = ntpb * P
    nst = -(-ntpb // ST)          # supertiles per block
    Sig = mybir.ActivationFunctionType.Sigmoid
    EQ = mybir.AluOpType.is_equal
    Wg = VG * VT
    ngrp = vpad // Wg
    do1 = phase in ("both", "p1")
    do2 = phase in ("both", "p2")

    # phase 1 (attention + segment reduce) and phase 2 (vocab scoring) are
    # software-pipelined: block b+1's phase-1 work is emitted in per-supertile
    # chunks between block b's phase-2 vocab groups, so the in-order engines
    # never expose phase-1 latency at block boundaries and the y-store stream
    # stays saturated.
    with tc.tile_pool(name="p1big", bufs=2) as pb, \
         tc.tile_pool(name="p1", bufs=3) as pool, \
         tc.tile_pool(name="blkp", bufs=2) as blkp, \
         tc.tile_pool(name="p2o", bufs=8) as p2o, \
         tc.tile_pool(name="psPre", bufs=2, space="PSUM") as psPre, \
         tc.tile_pool(name="psBc", bufs=1, space="PSUM") as psBc, \
         tc.tile_pool(name="psS", bufs=1, space="PSUM") as psS, \
         tc.tile_pool(name="psAl", bufs=1, space="PSUM") as psAl, \
         tc.tile_pool(name="ps2", bufs=3, space="PSUM") as ps2:

        def p1_segments(blk):
            """Emit phase 1 for `blk` as a list of closures (prologue,
            one per supertile, epilogue) sharing state via `sv`."""
            gsl = slice(blk * P, (blk + 1) * P)
            nsl = slice(blk * npb, (blk + 1) * npb)
            sv = {}
            n_mm = sum(min(ST, ntpb - ST * s) for s in range(nst))

            def prologue():
                sv["psS_t"] = psS.tile([P, 4 * P], f32, tag="psS", name="psS_t",
                                       space="PSUM")
                xpk = pb.tile([P, npb], bf16, tag="xpk")
                nc.sync.dma_start(out=xpk[:], in_=xpk_ext[:, nsl])
                xtp = pb.tile([P, npb], bf16, tag="xtp")
                nc.sync.dma_start(out=xtp[:], in_=xtp_ext[:, nsl])
                sv["xpk"], sv["xtp"] = xpk, xtp
                # q1g[g, d] = (v_n_blk @ W1_w.T)[g, d] (biases folded later)
                q1g_ps = sv["psS_t"][:, 0:P]
                nc.tensor.matmul(out=q1g_ps, lhsT=vnt[:, gsl], rhs=w1t[:],
                                 start=True, stop=True)
                q1g = blkp.tile([P, P], bf16, tag="q1g")
                nc.vector.tensor_copy(out=q1g[:], in_=q1g_ps)
                sv["q1g"] = q1g
                sv["mm_i"] = 0

            def supertile(st):
                blc = blc_sb[blk]
                q1g, xpk, xtp = sv["q1g"], sv["xpk"], sv["xtp"]
                sg_ps = sv["psS_t"][:, P:2 * P]
                nsub = min(ST, ntpb - ST * st)
                w = nsub * P
                ssl = slice(st * ST * P, st * ST * P + w)  # cols in block
                # batchloc replicated down partitions via K=4 row-select
                bc_ps = psBc.tile([P, ST * P], f32, tag="bc", space="PSUM")
                nc.tensor.matmul(out=bc_ps[:, :w],
                                 lhsT=sel4[:, blk * P:(blk + 1) * P],
                                 rhs=blr4[:, ssl], start=True, stop=True)
                # S^T[g, n] = (batchloc[n] == g)   [one op, 512 wide]
                StT = pool.tile([P, ST * P], bf16, tag="StT")
                nc.vector.tensor_scalar(out=StT[:, :w], in0=bc_ps[:, :w],
                                        scalar1=iota_col[:], scalar2=None,
                                        op0=EQ)
                # S[n, g] per 128-node subtile
                S_st = pool.tile([P, ST * P], bf16, tag="S")
                for c in range(nsub):
                    csl = slice(c * P, (c + 1) * P)
                    nc.vector.tensor_scalar(
                        out=S_st[:, csl], in0=iota_row[:],
                        scalar1=blc[:, st * ST + c:st * ST + c + 1],
                        scalar2=None, op0=EQ)
                # pre^T[d, n] = W2 @ x^T + q1g^T-expand   (+b12 in ACT)
                pre_ps = psPre.tile([P, ST * P], f32, tag="pre", space="PSUM")
                nc.tensor.matmul(out=pre_ps[:, :w], lhsT=w2t[:],
                                 rhs=xtp[:, ssl], start=True, stop=False)
                nc.tensor.matmul(out=pre_ps[:, :w], lhsT=q1g[:],
                                 rhs=StT[:, :w], start=False, stop=True)
                sigT = pool.tile([P, ST * P], bf16, tag="sigT")
                nc.scalar.activation(out=sigT[:, :w], in_=pre_ps[:, :w],
                                     func=Sig, bias=b12c[:])
                # alpha[n] = sig @ q_w.T (+ q_b in copy)
                al_ps = psAl.tile([P, ST], f32, tag="al", space="PSUM")
                for c in range(nsub):
                    csl = slice(c * P, (c + 1) * P)
                    nc.tensor.matmul(out=al_ps[:, c:c + 1],
                                     lhsT=sigT[:, csl], rhs=qwt[:],
                                     start=True, stop=True)
                al = pool.tile([P, ST], f32, tag="al")
                nc.vector.tensor_scalar_add(out=al[:, :nsub],
                                            in0=al_ps[:, :nsub],
                                            scalar1=qbc[:])
                # xa = alpha * x ; s_g^T[d, g] += xa^T-reduce via S
                xa = pool.tile([P, ST * P], bf16, tag="xa")
                for c in range(nsub):
                    csl = slice(c * P, (c + 1) * P)
                    nc.vector.tensor_scalar_mul(
                        out=xa[:, csl],
                        in0=xpk[:, st * ST * P + c * P:st * ST * P + (c + 1) * P],
                        scalar1=al[:, c:c + 1])
                    nc.tensor.matmul(out=sg_ps, lhsT=xa[:, csl],
                                     rhs=S_st[:, csl],
                                     start=(sv["mm_i"] == 0),
                                     stop=(sv["mm_i"] == n_mm - 1))
                    sv["mm_i"] += 1

            def epilogue():
                sg_ps = sv["psS_t"][:, P:2 * P]
                sg_sb = blkp.tile([P, P], f32, tag="sgsb")
                nc.vector.tensor_copy(out=sg_sb[:], in_=sg_ps)
                # s_h^T[d, g] = W3a @ v_n^T + W3b @ s_g^T  (+W3_b in copy)
                sh_ps = sv["psS_t"][:, 0:P]
                nc.tensor.matmul(out=sh_ps, lhsT=w3at[:], rhs=vnt[:, gsl],
                                 start=True, stop=False)
                nc.tensor.matmul(out=sh_ps, lhsT=w3bt[:], rhs=sg_sb[:],
                                 start=False, stop=True)
                nc.vector.tensor_scalar_add(out=shT[:, gsl], in0=sh_ps,
                                            scalar1=w3bc[:])

            return ([prologue]
                    + [lambda st=st: supertile(st) for st in range(nst)]
                    + [epilogue])

        def p2_group(blk, g):
            gsl = slice(blk * P, (blk + 1) * P)
            gvsl = slice(g * Wg, (g + 1) * Wg)
            itw = itw_pre[g]
            sc = p2o.tile([P, Wg], bf16, tag="scsb")
            for s in range(VG):
                sc_ps = ps2.tile([P, VT], f32, tag="sc", space="PSUM")
                nc.tensor.matmul(out=sc_ps[:],
                                 lhsT=shT[:, gsl],
                                 rhs=itw[:, s * VT:(s + 1) * VT],
                                 start=True, stop=True)
                if s % 2 == 0:
                    nc.vector.tensor_copy(out=sc[:, s * VT:(s + 1) * VT],
                                          in_=sc_ps[:])
                else:
                    nc.scalar.copy(out=sc[:, s * VT:(s + 1) * VT],
                                   in_=sc_ps[:])
            nc.sync.dma_start(out=y_ext[blk * P:(blk + 1) * P, gvsl],
                              in_=sc[:])

        if not do2:
            for blk in range(NBLK):
                for seg in p1_segments(blk):
                    seg()
            return
        if do1:
            for seg in p1_segments(0):
                seg()
        for blk in range(NBLK):
            # interleave next block's phase-1 chunks between vocab groups,
            # starting a couple of groups in so the store queue fills first
            nxt = list(p1_segments(blk + 1)) if (do1 and blk + 1 < NBLK) else []
            for g in range(ngrp):
                p2_group(blk, g)
                if nxt and g >= 1:
                    nxt.pop(0)()
                    if len(nxt) > ngrp - g - 2:   # don't leave a tail
                        nxt.pop(0)()
            while nxt:
                nxt.pop(0)()


def prep_inputs(session_embedding, item_weight, W1_w, W1_b, W2_w, W2_b,
                q_w, q_b, W3_w, W3_b, batch, num_graphs):
    """Host-side sharding/layout. Returns (in_maps, ntpb, vpad, V, cc)."""
    x = np.ascontiguousarray(np.asarray(session_embedding, dtype=np.float32))
    itw = np.asarray(item_weight, dtype=np.float32)
    batch = np.asarray(batch).astype(np.int64)
    B = int(num_graphs)
    N, d = x.shape
    V = itw.shape[0]
    assert d == D and B == NCORES * BC, (d, B)

    counts = np.bincount(batch, minlength=B)
    assert counts.min() >= 1, "every graph must be non-empty"
    starts = np.zeros(B + 1, np.int64)
    np.cumsum(counts, out=starts[1:])
    assert starts[-1] == N
    last_idx = starts[1:] - 1
    v_n = x[last_idx]                                   # [B, D]

    blk_cnt = starts[P::P] - starts[:-P:P].reshape(-1)  # [B//P]
    ntpb = int(-(-blk_cnt.max() // P))                  # ceil
    npb = ntpb * P

    vpad = -(-V // (VT * VG)) * (VT * VG)
    itwT = np.zeros((D, vpad), BF16)
    itwT[:, :V] = itw.T.astype(BF16)

    w1t = np.ascontiguousarray(np.asarray(W1_w, np.float32).T)
    w2t = np.ascontiguousarray(np.asarray(W2_w, np.float32).T.astype(BF16))
    W3 = np.asarray(W3_w, np.float32)
    w3at = np.ascontiguousarray(W3[:, :D].T)
    w3bt = np.ascontiguousarray(W3[:, D:].T)
    b12c = (np.asarray(W1_b, np.float32) + np.asarray(W2_b, np.float32)
            ).reshape(P, 1).copy()
    w3bc = np.asarray(W3_b, np.float32).reshape(P, 1).copy()
    qwt = np.ascontiguousarray(
        np.asarray(q_w, np.float32).reshape(1, D).T.astype(BF16))
    sel4 = np.zeros((NBLK, NBLK * P), BF16)
    for k in range(NBLK):
        sel4[k, k * P:(k + 1) * P] = 1
    qbc = np.full((P, 1), np.float32(np.asarray(q_b).reshape(())), np.float32)

    in_maps = []
    for c in range(NCORES):
        xpad = np.zeros((NBLK, npb, D), np.float32)
        bl = np.zeros((NBLK, P, ntpb), np.float32)
        blr = np.zeros((NBLK, npb), BF16)
        for b in range(NBLK):
            glo = c * BC + b * P
            s, e = int(starts[glo]), int(starts[glo + P])
            cnt = e - s
            assert cnt <= npb
            xpad[b, :cnt] = x[s:e]
            locp = np.zeros(npb, np.float32)
            locp[:cnt] = (batch[s:e] - glo).astype(np.float32)
            bl[b] = locp.reshape(ntpb, P).T
            blr[b] = locp.astype(BF16)
        # packed node-row: xpk[:, blk*npb + t*128 + j][i] = x_pad[blk, t*128+i, j]
        xpk = np.ascontiguousarray(
            xpad.reshape(NBLK, ntpb, P, D).transpose(2, 0, 1, 3)
            .reshape(P, NBLK * npb).astype(BF16))
        # feature-row transposed: xtp[:, blk*npb + n] = x_pad[blk, n, :]
        xtp = np.ascontiguousarray(
            xpad.transpose(2, 0, 1).reshape(P, NBLK * npb).astype(BF16))
        vnt = np.ascontiguousarray(v_n[c * BC:(c + 1) * BC].T)
        im = dict(
            xpk=xpk, xtp=xtp, blc=np.ascontiguousarray(bl),
            blr=np.ascontiguousarray(blr), vnt=vnt,
            w1t=w1t, w2t=w2t, w3at=w3at, w3bt=w3bt,
            b12c=b12c, w3bc=w3bc, qwt=qwt, qbc=qbc, sel4=sel4, itwt=itwT)
        in_maps.append(im)
    return in_maps, ntpb, vpad, V, False


_NC_CACHE = {}


def get_nc(ntpb, vpad, repeat=1, phase="both", cc=False):
    key = (ntpb, vpad, repeat, phase)
    if key not in _NC_CACHE:
        _NC_CACHE[key] = build_nc(ntpb, vpad, repeat, phase)
    return _NC_CACHE[key]


def kernel(**inputs):
    from concourse.bass_utils import run_bass_kernel_spmd

    in_maps, ntpb, vpad, V, cc = prep_inputs(**inputs)
    nc = get_nc(ntpb, vpad)
    res = run_bass_kernel_spmd(nc, in_maps, core_ids=list(range(NCORES)))
    B = int(inputs["num_graphs"])
    y = np.empty((B, V), np.float32)
    for c in range(NCORES):
        y[c * BC:(c + 1) * BC] = res.results[c]["y"][:, :V]
    return y


# revision 20
# speedup vs baseline: 2.9610x; 1.1041x over previous
"""Trainium2 Bass kernel for nn_Embedding2Score (segment_reduce).

Strategy (data-parallel over sessions, per sharding hint):
  - 4096 graphs -> 8 cores x 512 graphs (4 blocks of 128 graphs each).
    Each core owns whole contiguous segments (batch is sorted by graph).
  - Nodes are processed in 512-node supertiles (4 x 128-node subtiles).
    Segment broadcast (v_n -> nodes) and segment sum (alpha*x -> s_g) are
    one-hot matmuls on PE; one-hot blocks are built with DVE is_equal.
    The graph-major one-hot S^T needs batchloc replicated down the 128
    partitions: done with a K=1 ones-matmul into PSUM (no DMA broadcast).
  - bf16 data path: x (both layouts), item_weight, W2, q_w and all one-hot
    matrices are bf16 (PE runs 1 cycle/row vs 4 for fp32, DMA bytes halve).
    Accumulation stays fp32 in PSUM; biases/alpha/v_n/W1/W3 stay fp32.
    rel-err budget is 2e-2; bf16 keeps it ~2e-3.
  - Final scoring: s_h^T [128d, 512g] per core vs item_weight^T tiles,
    grouped 4 vocab-tiles per DMA so all phase-2 DMAs are ~1 MB.
    Output rows [512, V] per core = row-slice of [4096, 50000].
"""

import sys

if "/opt/trn_rl_repo" not in sys.path:
    sys.path.insert(0, "/opt/trn_rl_repo")

import numpy as np
import ml_dtypes

BF16 = ml_dtypes.bfloat16
P = 128          # partitions / tile edge
D = 128          # hidden size
NCORES = 8
NBLK = 4         # graph blocks per core, 128 graphs each
BC = NBLK * P    # graphs per core = 512
VT = 500         # vocab tile (psum bank holds 512 fp32)
VG = 4           # vocab tiles per DMA group
ST = 4           # 128-node subtiles per supertile
NPF = 25         # vocab groups prefetched (all SBUF-resident)


def build_nc(ntpb, vpad, repeat=1, phase="both"):
    """Build the per-core Bass program. ntpb = node tiles per graph-block,
    vpad = padded vocab size (multiple of VT*VG). repeat>1 wraps the body
    in a hardware loop (timing probes). phase: 'both' | 'p1' | 'p2'."""
    import contextlib
    import concourse.bacc as bacc
    import concourse.mybir as mybir
    from concourse.tile import TileContext

    f32 = mybir.dt.float32
    bf16 = mybir.dt.bfloat16
    npb = ntpb * P
    nc = bacc.Bacc()

    xpk_ext = nc.declare_dram_parameter("xpk", [P, NBLK * npb], bf16, isOutput=False)
    xtp_ext = nc.declare_dram_parameter("xtp", [P, NBLK * npb], bf16, isOutput=False)
    blc_ext = nc.declare_dram_parameter("blc", [NBLK, P, ntpb], f32, isOutput=False)
    blr_ext = nc.declare_dram_parameter("blr", [NBLK, npb], bf16, isOutput=False)
    vnt_ext = nc.declare_dram_parameter("vnt", [D, BC], f32, isOutput=False)
    w1t_ext = nc.declare_dram_parameter("w1t", [D, D], f32, isOutput=False)
    w2t_ext = nc.declare_dram_parameter("w2t", [D, D], bf16, isOutput=False)
    w3at_ext = nc.declare_dram_parameter("w3at", [D, D], f32, isOutput=False)
    w3bt_ext = nc.declare_dram_parameter("w3bt", [D, D], f32, isOutput=False)
    b12c_ext = nc.declare_dram_parameter("b12c", [P, 1], f32, isOutput=False)
    w3bc_ext = nc.declare_dram_parameter("w3bc", [P, 1], f32, isOutput=False)
    qwt_ext = nc.declare_dram_parameter("qwt", [D, 1], bf16, isOutput=False)
    sel4_ext = nc.declare_dram_parameter("sel4", [NBLK, NBLK * P], bf16,
                                         isOutput=False)
    qbc_ext = nc.declare_dram_parameter("qbc", [P, 1], f32, isOutput=False)
    itwt_ext = nc.declare_dram_parameter("itwt", [D, vpad], bf16, isOutput=False)
    y_ext = nc.declare_dram_parameter("y", [BC, vpad], bf16, isOutput=True)

    with TileContext(nc) as tc:
        with tc.tile_pool(name="const", bufs=1) as cp:
            iota_i = cp.tile([P, P], mybir.dt.int32, tag="iotai")
            nc.gpsimd.iota(iota_i[:], pattern=[[1, P]], base=0, channel_multiplier=0)
            iota_f = cp.tile([P, P], f32, tag="iotaf")
            nc.vector.tensor_copy(out=iota_f[:], in_=iota_i[:])
            iota_row = cp.tile([P, P], bf16, tag="iotarow")
            nc.vector.tensor_copy(out=iota_row[:], in_=iota_f[:])
            iota_ci = cp.tile([P, 1], mybir.dt.int32, tag="iotaci")
            nc.gpsimd.iota(iota_ci[:], pattern=[[0, 1]], base=0, channel_multiplier=1)
            iota_col = cp.tile([P, 1], f32, tag="iotacol")
            nc.vector.tensor_copy(out=iota_col[:], in_=iota_ci[:])
            ones_row = cp.tile([1, P], bf16, tag="onesrow")
            nc.vector.memset(ones_row[:], 1.0)

            def load(name, ext, shape, dt):
                t = cp.tile(shape, dt, tag=name)
                nc.sync.dma_start(out=t[:], in_=ext[:])
                return t

            w1t = load("w1t", w1t_ext, [D, D], f32)
            w2t = load("w2t", w2t_ext, [D, D], bf16)
            w3at = load("w3at", w3at_ext, [D, D], f32)
            w3bt = load("w3bt", w3bt_ext, [D, D], f32)
            b12c = load("b12c", b12c_ext, [P, 1], f32)
            w3bc = load("w3bc", w3bc_ext, [P, 1], f32)
            qwt = load("qwt", qwt_ext, [D, 1], bf16)
            qbc = load("qbc", qbc_ext, [P, 1], f32)
            vnt = load("vnt", vnt_ext, [D, BC], f32)

            shT = cp.tile([D, BC], bf16, tag="shT")  # s_h^T, filled per block
            if phase == "p2":
                nc.vector.memset(shT[:], 0.01)
            # per-block batchloc tables are iteration-invariant: load once
            blc_sb = []
            for blk in range(NBLK):
                t = cp.tile([P, ntpb], f32, tag=f"blc{blk}")
                nc.sync.dma_start(out=t[:], in_=blc_ext[blk])
                blc_sb.append(t)
            blr4 = cp.tile([NBLK, npb], bf16, tag="blr4")
            nc.sync.dma_start(out=blr4[:], in_=blr_ext[:])
            sel4 = cp.tile([NBLK, NBLK * P], bf16, tag="sel4")
            nc.sync.dma_start(out=sel4[:], in_=sel4_ext[:])
            itw_pre = []
            W = VG * VT
            for g in range(min(NPF, vpad // W)):
                t = cp.tile([D, W], bf16, tag=f"itwpre{g}")
                nc.sync.dma_start(out=t[:], in_=itwt_ext[:, g * W:(g + 1) * W])
                itw_pre.append(t)

            _build_body(nc, tc, mybir, ntpb, vpad,
                        xpk_ext, xtp_ext, blc_sb, blr4, sel4, itwt_ext, y_ext,
                        iota_row, iota_col, ones_row,
                        w1t, w2t, w3at, w3bt, b12c, w3bc, qwt, qbc, vnt, shT,
                        phase, itw_pre, repeat)

    nc.compile()
    return nc


def _build_body(nc, tc, mybir, ntpb, vpad,
                xpk_ext, xtp_ext, blc_sb, blr4, sel4, itwt_ext, y_ext,
                iota_row, iota_col, ones_row,
                w1t, w2t, w3at, w3bt, b12c, w3bc, qwt, qbc, vnt, shT,
                phase="both", itw_pre=None, repeat=1):
    import contextlib
    f32 = mybir.dt.float32
    bf16 = mybir.dt.bfloat16
    npb = ntpb * P
    nst = -(-ntpb // ST)          # supertiles per block
    Sig = mybir.ActivationFunctionType.Sigmoid
    EQ = mybir.AluOpType.is_equal
    Wg = VG * VT
    ngrp = vpad // Wg
    do1 = phase in ("both", "p1")
    do2 = phase in ("both", "p2")

    # phase 1 (attention + segment reduce) and phase 2 (vocab scoring) are
    # software-pipelined: block b+1's phase-1 work is emitted in per-supertile
    # chunks between block b's phase-2 vocab groups, so the in-order engines
    # never expose phase-1 latency at block boundaries and the y-store stream
    # stays saturated.
    with tc.tile_pool(name="p1big", bufs=2) as pb, \
         tc.tile_pool(name="p1", bufs=3) as pool, \
         tc.tile_pool(name="blkp", bufs=2) as blkp, \
         tc.tile_pool(name="p2o", bufs=8) as p2o, \
         tc.tile_pool(name="psPre", bufs=2, space="PSUM") as psPre, \
         tc.tile_pool(name="psBc", bufs=1, space="PSUM") as psBc, \
         tc.tile_pool(name="psS", bufs=1, space="PSUM") as psS, \
         tc.tile_pool(name="psAl", bufs=1, space="PSUM") as psAl, \
         tc.tile_pool(name="ps2", bufs=3, space="PSUM") as ps2:

        def p1_segments(blk):
            """Emit phase 1 for `blk` as a list of closures (prologue,
            one per supertile, epilogue) sharing state via `sv`."""
            gsl = slice(blk * P, (blk + 1) * P)
            nsl = slice(blk * npb, (blk + 1) * npb)
            sv = {}
            n_mm = sum(min(ST, ntpb - ST * s) for s in range(nst))

            def prologue():
                sv["psS_t"] = psS.tile([P, 4 * P], f32, tag="psS", name="psS_t",
                                       space="PSUM")
                xpk = pb.tile([P, npb], bf16, tag="xpk")
                nc.sync.dma_start(out=xpk[:], in_=xpk_ext[:, nsl])
                xtp = pb.tile([P, npb], bf16, tag="xtp")
                nc.sync.dma_start(out=xtp[:], in_=xtp_ext[:, nsl])
                sv["xpk"], sv["xtp"] = xpk, xtp
                # q1g[g, d] = (v_n_blk @ W1_w.T)[g, d] (biases folded later)
                q1g_ps = sv["psS_t"][:, 0:P]
                nc.tensor.matmul(out=q1g_ps, lhsT=vnt[:, gsl], rhs=w1t[:],
                                 start=True, stop=True)
                q1g = blkp.tile([P, P], bf16, tag="q1g")
                nc.vector.tensor_copy(out=q1g[:], in_=q1g_ps)
                sv["q1g"] = q1g
                sv["mm_i"] = 0

            def supertile(st):
                blc = blc_sb[blk]
                q1g, xpk, xtp = sv["q1g"], sv["xpk"], sv["xtp"]
                sg_ps = sv["psS_t"][:, P:2 * P]
                nsub = min(ST, ntpb - ST * st)
                w = nsub * P
                ssl = slice(st * ST * P, st * ST * P + w)  # cols in block
                # batchloc replicated down partitions via K=4 row-select
                bc_ps = psBc.tile([P, ST * P], f32, tag="bc", space="PSUM")
                nc.tensor.matmul(out=bc_ps[:, :w],
                                 lhsT=sel4[:, blk * P:(blk + 1) * P],
                                 rhs=blr4[:, ssl], start=True, stop=True)
                # S^T[g, n] = (batchloc[n] == g)   [one op, 512 wide]
                StT = pool.tile([P, ST * P], bf16, tag="StT")
                nc.vector.tensor_scalar(out=StT[:, :w], in0=bc_ps[:, :w],
                                        scalar1=iota_col[:], scalar2=None,
                                        op0=EQ)
                # S[n, g] per 128-node subtile
                S_st = pool.tile([P, ST * P], bf16, tag="S")
                for c in range(nsub):
                    csl = slice(c * P, (c + 1) * P)
                    nc.vector.tensor_scalar(
                        out=S_st[:, csl], in0=iota_row[:],
                        scalar1=blc[:, st * ST + c:st * ST + c + 1],
                        scalar2=None, op0=EQ)
                # pre^T[d, n] = W2 @ x^T + q1g^T-expand   (+b12 in ACT)
                pre_ps = psPre.tile([P, ST * P], f32, tag="pre", space="PSUM")
                nc.tensor.matmul(out=pre_ps[:, :w], lhsT=w2t[:],
                                 rhs=xtp[:, ssl], start=True, stop=False)
                nc.tensor.matmul(out=pre_ps[:, :w], lhsT=q1g[:],
                                 rhs=StT[:, :w], start=False, stop=True)
                sigT = pool.tile([P, ST * P], bf16, tag="sigT")
                nc.scalar.activation(out=sigT[:, :w], in_=pre_ps[:, :w],
                                     func=Sig, bias=b12c[:])
                # alpha[n] = sig @ q_w.T (+ q_b in copy)
                al_ps = psAl.tile([P, ST], f32, tag="al", space="PSUM")
                for c in range(nsub):
                    csl = slice(c * P, (c + 1) * P)
                    nc.tensor.matmul(out=al_ps[:, c:c + 1],
                                     lhsT=sigT[:, csl], rhs=qwt[:],
                                     start=True, stop=True)
                al = pool.tile([P, ST], f32, tag="al")
                nc.vector.tensor_scalar_add(out=al[:, :nsub],
                                            in0=al_ps[:, :nsub],
                                            scalar1=qbc[:])
                # xa = alpha * x ; s_g^T[d, g] += xa^T-reduce via S
                xa = pool.tile([P, ST * P], bf16, tag="xa")
                for c in range(nsub):
                    csl = slice(c * P, (c + 1) * P)
                    nc.vector.tensor_scalar_mul(
                        out=xa[:, csl],
                        in0=xpk[:, st * ST * P + c * P:st * ST * P + (c + 1) * P],
                        scalar1=al[:, c:c + 1])
                    nc.tensor.matmul(out=sg_ps, lhsT=xa[:, csl],
                                     rhs=S_st[:, csl],
                                     start=(sv["mm_i"] == 0),
                                     stop=(sv["mm_i"] == n_mm - 1))
                    sv["mm_i"] += 1

            def epilogue():
                sg_ps = sv["psS_t"][:, P:2 * P]
                sg_sb = blkp.tile([P, P], f32, tag="sgsb")
                nc.vector.tensor_copy(out=sg_sb[:], in_=sg_ps)
                # s_h^T[d, g] = W3a @ v_n^T + W3b @ s_g^T  (+W3_b in copy)
                sh_ps = sv["psS_t"][:, 0:P]
                nc.tensor.matmul(out=sh_ps, lhsT=w3at[:], rhs=vnt[:, gsl],
                                 start=True, stop=False)
                nc.tensor.matmul(out=sh_ps, lhsT=w3bt[:], rhs=sg_sb[:],
                                 start=False, stop=True)
                nc.vector.tensor_scalar_add(out=shT[:, gsl], in0=sh_ps,
                                            scalar1=w3bc[:])

            return ([prologue]
                    + [lambda st=st: supertile(st) for st in range(nst)]
                    + [epilogue])

        def p2_group(blk, g):
            gsl = slice(blk * P, (blk + 1) * P)
            gvsl = slice(g * Wg, (g + 1) * Wg)
            itw = itw_pre[g]
            sc = p2o.tile([P, Wg], bf16, tag="scsb")
            for s in range(VG):
                sc_ps = ps2.tile([P, VT], f32, tag="sc", space="PSUM")
                nc.tensor.matmul(out=sc_ps[:],
                                 lhsT=shT[:, gsl],
                                 rhs=itw[:, s * VT:(s + 1) * VT],
                                 start=True, stop=True)
                if (g * VG + s) % 8 < 3:
                    nc.vector.tensor_copy(out=sc[:, s * VT:(s + 1) * VT],
                                          in_=sc_ps[:])
                else:
                    nc.scalar.copy(out=sc[:, s * VT:(s + 1) * VT],
                                   in_=sc_ps[:])
            nc.sync.dma_start(out=y_ext[blk * P:(blk + 1) * P, gvsl],
                              in_=sc[:])

        if not do2:
            rep_ctx = (tc.For_i(0, repeat, 1) if repeat > 1
                       else contextlib.nullcontext())
            with rep_ctx:
                for blk in range(NBLK):
                    for seg in p1_segments(blk):
                        seg()
            return
        if do1:
            # prologue: block 0's phase 1 runs once before the loop; inside
            # the loop each iteration computes block 0's (identical) phase 1
            # for the NEXT iteration under block 3's store stream.
            for seg in p1_segments(0):
                seg()
        rep_ctx = (tc.For_i(0, repeat, 1) if repeat > 1
                   else contextlib.nullcontext())
        with rep_ctx:
            for blk in range(NBLK):
                nxt = list(p1_segments((blk + 1) % NBLK)) if do1 else []
                for g in range(ngrp):
                    p2_group(blk, g)
                    if nxt and g >= 1:
                        nxt.pop(0)()
                        if len(nxt) > ngrp - g - 2:   # don't leave a tail
                            nxt.pop(0)()
                while nxt:
                    nxt.pop(0)()


def prep_inputs(session_embedding, item_weight, W1_w, W1_b, W2_w, W2_b,
                q_w, q_b, W3_w, W3_b, batch, num_graphs):
    """Host-side sharding/layout. Returns (in_maps, ntpb, vpad, V, cc)."""
    x = np.ascontiguousarray(np.asarray(session_embedding, dtype=np.float32))
    itw = np.asarray(item_weight, dtype=np.float32)
    batch = np.asarray(batch).astype(np.int64)
    B = int(num_graphs)
    N, d = x.shape
    V = itw.shape[0]
    assert d == D and B == NCORES * BC, (d, B)

    counts = np.bincount(batch, minlength=B)
    assert counts.min() >= 1, "every graph must be non-empty"
    starts = np.zeros(B + 1, np.int64)
    np.cumsum(counts, out=starts[1:])
    assert starts[-1] == N
    last_idx = starts[1:] - 1
    v_n = x[last_idx]                                   # [B, D]

    blk_cnt = starts[P::P] - starts[:-P:P].reshape(-1)  # [B//P]
    ntpb = int(-(-blk_cnt.max() // P))                  # ceil
    npb = ntpb * P

    vpad = -(-V // (VT * VG)) * (VT * VG)
    itwT = np.zeros((D, vpad), BF16)
    itwT[:, :V] = itw.T.astype(BF16)

    w1t = np.ascontiguousarray(np.asarray(W1_w, np.float32).T)
    w2t = np.ascontiguousarray(np.asarray(W2_w, np.float32).T.astype(BF16))
    W3 = np.asarray(W3_w, np.float32)
    w3at = np.ascontiguousarray(W3[:, :D].T)
    w3bt = np.ascontiguousarray(W3[:, D:].T)
    b12c = (np.asarray(W1_b, np.float32) + np.asarray(W2_b, np.float32)
            ).reshape(P, 1).copy()
    w3bc = np.asarray(W3_b, np.float32).reshape(P, 1).copy()
    qwt = np.ascontiguousarray(
        np.asarray(q_w, np.float32).reshape(1, D).T.astype(BF16))
    sel4 = np.zeros((NBLK, NBLK * P), BF16)
    for k in range(NBLK):
        sel4[k, k * P:(k + 1) * P] = 1
    qbc = np.full((P, 1), np.float32(np.asarray(q_b).reshape(())), np.float32)

    in_maps = []
    for c in range(NCORES):
        xpad = np.zeros((NBLK, npb, D), np.float32)
        bl = np.zeros((NBLK, P, ntpb), np.float32)
        blr = np.zeros((NBLK, npb), BF16)
        for b in range(NBLK):
            glo = c * BC + b * P
            s, e = int(starts[glo]), int(starts[glo + P])
            cnt = e - s
            assert cnt <= npb
            xpad[b, :cnt] = x[s:e]
            locp = np.zeros(npb, np.float32)
            locp[:cnt] = (batch[s:e] - glo).astype(np.float32)
            bl[b] = locp.reshape(ntpb, P).T
            blr[b] = locp.astype(BF16)
        # packed node-row: xpk[:, blk*npb + t*128 + j][i] = x_pad[blk, t*128+i, j]
        xpk = np.ascontiguousarray(
            xpad.reshape(NBLK, ntpb, P, D).transpose(2, 0, 1, 3)
            .reshape(P, NBLK * npb).astype(BF16))
        # feature-row transposed: xtp[:, blk*npb + n] = x_pad[blk, n, :]
        xtp = np.ascontiguousarray(
            xpad.transpose(2, 0, 1).reshape(P, NBLK * npb).astype(BF16))
        vnt = np.ascontiguousarray(v_n[c * BC:(c + 1) * BC].T)
        im = dict(
            xpk=xpk, xtp=xtp, blc=np.ascontiguousarray(bl),
            blr=np.ascontiguousarray(blr), vnt=vnt,
            w1t=w1t, w2t=w2t, w3at=w3at, w3bt=w3bt,
            b12c=b12c, w3bc=w3bc, qwt=qwt, qbc=qbc, sel4=sel4, itwt=itwT)
        in_maps.append(im)
    return in_maps, ntpb, vpad, V, False


_NC_CACHE = {}


def get_nc(ntpb, vpad, repeat=1, phase="both", cc=False):
    key = (ntpb, vpad, repeat, phase)
    if key not in _NC_CACHE:
        _NC_CACHE[key] = build_nc(ntpb, vpad, repeat, phase)
    return _NC_CACHE[key]


def kernel(**inputs):
    from concourse.bass_utils import run_bass_kernel_spmd

    in_maps, ntpb, vpad, V, cc = prep_inputs(**inputs)
    nc = get_nc(ntpb, vpad)
    res = run_bass_kernel_spmd(nc, in_maps, core_ids=list(range(NCORES)))
    B = int(inputs["num_graphs"])
    y = np.empty((B, V), np.float32)
    for c in range(NCORES):
        y[c * BC:(c + 1) * BC] = res.results[c]["y"][:, :V]
    return y


# revision 22
# speedup vs baseline: 3.0184x; 1.0194x over previous
"""Trainium2 Bass kernel for nn_Embedding2Score (segment_reduce).

Strategy (data-parallel over sessions, per sharding hint):
  - 4096 graphs -> 8 cores x 512 graphs (4 blocks of 128 graphs each).
    Each core owns whole contiguous segments (batch is sorted by graph).
  - Nodes are processed in 512-node supertiles (4 x 128-node subtiles).
    Segment broadcast (v_n -> nodes) and segment sum (alpha*x -> s_g) are
    one-hot matmuls on PE; one-hot blocks are built with DVE is_equal.
    The graph-major one-hot S^T needs batchloc replicated down the 128
    partitions: done with a K=1 ones-matmul into PSUM (no DMA broadcast).
  - bf16 data path: x (both layouts), item_weight, W2, q_w and all one-hot
    matrices are bf16 (PE runs 1 cycle/row vs 4 for fp32, DMA bytes halve).
    Accumulation stays fp32 in PSUM; biases/alpha/v_n/W1/W3 stay fp32.
    rel-err budget is 2e-2; bf16 keeps it ~2e-3.
  - Final scoring: s_h^T [128d, 512g] per core vs item_weight^T tiles,
    grouped 4 vocab-tiles per DMA so all phase-2 DMAs are ~1 MB.
    Output rows [512, V] per core = row-slice of [4096, 50000].
"""

import sys

if "/opt/trn_rl_repo" not in sys.path:
    sys.path.insert(0, "/opt/trn_rl_repo")

import numpy as np
import ml_dtypes

BF16 = ml_dtypes.bfloat16
P = 128          # partitions / tile edge
D = 128          # hidden size
NCORES = 8
NBLK = 4         # graph blocks per core, 128 graphs each
BC = NBLK * P    # graphs per core = 512
VT = 500         # vocab tile (psum bank holds 512 fp32)
VG = 4           # vocab tiles per DMA group
ST = 4           # 128-node subtiles per supertile
NPF = 25         # vocab groups prefetched (all SBUF-resident)


def build_nc(ntpb, vpad, repeat=1, phase="both"):
    """Build the per-core Bass program. ntpb = node tiles per graph-block,
    vpad = padded vocab size (multiple of VT*VG). repeat>1 wraps the body
    in a hardware loop (timing probes). phase: 'both' | 'p1' | 'p2'."""
    import contextlib
    import concourse.bacc as bacc
    import concourse.mybir as mybir
    from concourse.tile import TileContext

    f32 = mybir.dt.float32
    bf16 = mybir.dt.bfloat16
    npb = ntpb * P
    nc = bacc.Bacc()

    xpk_ext = nc.declare_dram_parameter("xpk", [P, NBLK * npb], bf16, isOutput=False)
    xtp_ext = nc.declare_dram_parameter("xtp", [P, NBLK * npb], bf16, isOutput=False)
    blc_ext = nc.declare_dram_parameter("blc", [NBLK, P, ntpb], f32, isOutput=False)
    blr_ext = nc.declare_dram_parameter("blr", [NBLK, npb], bf16, isOutput=False)
    vnt_ext = nc.declare_dram_parameter("vnt", [D, BC], f32, isOutput=False)
    w1t_ext = nc.declare_dram_parameter("w1t", [D, D], f32, isOutput=False)
    w2t_ext = nc.declare_dram_parameter("w2t", [D, D], bf16, isOutput=False)
    w3at_ext = nc.declare_dram_parameter("w3at", [D, D], f32, isOutput=False)
    w3bt_ext = nc.declare_dram_parameter("w3bt", [D, D], f32, isOutput=False)
    b12c_ext = nc.declare_dram_parameter("b12c", [P, 1], f32, isOutput=False)
    w3bc_ext = nc.declare_dram_parameter("w3bc", [P, 1], f32, isOutput=False)
    qwt_ext = nc.declare_dram_parameter("qwt", [D, 1], bf16, isOutput=False)
    sel4_ext = nc.declare_dram_parameter("sel4", [NBLK, NBLK * P], bf16,
                                         isOutput=False)
    qbc_ext = nc.declare_dram_parameter("qbc", [P, 1], f32, isOutput=False)
    itwt_ext = nc.declare_dram_parameter("itwt", [D, vpad], bf16, isOutput=False)
    y_ext = nc.declare_dram_parameter("y", [BC, vpad], bf16, isOutput=True)

    with TileContext(nc) as tc:
        with tc.tile_pool(name="const", bufs=1) as cp:
            iota_i = cp.tile([P, P], mybir.dt.int32, tag="iotai")
            nc.gpsimd.iota(iota_i[:], pattern=[[1, P]], base=0, channel_multiplier=0)
            iota_f = cp.tile([P, P], f32, tag="iotaf")
            nc.vector.tensor_copy(out=iota_f[:], in_=iota_i[:])
            iota_row = cp.tile([P, P], bf16, tag="iotarow")
            nc.vector.tensor_copy(out=iota_row[:], in_=iota_f[:])
            iota_ci = cp.tile([P, 1], mybir.dt.int32, tag="iotaci")
            nc.gpsimd.iota(iota_ci[:], pattern=[[0, 1]], base=0, channel_multiplier=1)
            iota_col = cp.tile([P, 1], f32, tag="iotacol")
            nc.vector.tensor_copy(out=iota_col[:], in_=iota_ci[:])
            ones_row = cp.tile([1, P], bf16, tag="onesrow")
            nc.vector.memset(ones_row[:], 1.0)

            def load(name, ext, shape, dt):
                t = cp.tile(shape, dt, tag=name)
                nc.sync.dma_start(out=t[:], in_=ext[:])
                return t

            w1t = load("w1t", w1t_ext, [D, D], f32)
            w2t = load("w2t", w2t_ext, [D, D], bf16)
            w3at = load("w3at", w3at_ext, [D, D], f32)
            w3bt = load("w3bt", w3bt_ext, [D, D], f32)
            b12c = load("b12c", b12c_ext, [P, 1], f32)
            w3bc = load("w3bc", w3bc_ext, [P, 1], f32)
            qwt = load("qwt", qwt_ext, [D, 1], bf16)
            qbc = load("qbc", qbc_ext, [P, 1], f32)
            vnt = load("vnt", vnt_ext, [D, BC], f32)

            shT = cp.tile([D, BC], bf16, tag="shT")  # s_h^T, filled per block
            if phase == "p2":
                nc.vector.memset(shT[:], 0.01)
            # per-block batchloc tables are iteration-invariant: load once
            blc_sb = []
            for blk in range(NBLK):
                t = cp.tile([P, ntpb], f32, tag=f"blc{blk}")
                nc.sync.dma_start(out=t[:], in_=blc_ext[blk])
                blc_sb.append(t)
            blr4 = cp.tile([NBLK, npb], bf16, tag="blr4")
            nc.sync.dma_start(out=blr4[:], in_=blr_ext[:])
            sel4 = cp.tile([NBLK, NBLK * P], bf16, tag="sel4")
            nc.sync.dma_start(out=sel4[:], in_=sel4_ext[:])
            itw_pre = []
            W = VG * VT
            for g in range(min(NPF, vpad // W)):
                t = cp.tile([D, W], bf16, tag=f"itwpre{g}")
                nc.sync.dma_start(out=t[:], in_=itwt_ext[:, g * W:(g + 1) * W])
                itw_pre.append(t)

            _build_body(nc, tc, mybir, ntpb, vpad,
                        xpk_ext, xtp_ext, blc_sb, blr4, sel4, itwt_ext, y_ext,
                        iota_row, iota_col, ones_row,
                        w1t, w2t, w3at, w3bt, b12c, w3bc, qwt, qbc, vnt, shT,
                        phase, itw_pre, repeat)

    nc.compile()
    return nc


def _build_body(nc, tc, mybir, ntpb, vpad,
                xpk_ext, xtp_ext, blc_sb, blr4, sel4, itwt_ext, y_ext,
                iota_row, iota_col, ones_row,
                w1t, w2t, w3at, w3bt, b12c, w3bc, qwt, qbc, vnt, shT,
                phase="both", itw_pre=None, repeat=1):
    import contextlib
    f32 = mybir.dt.float32
    bf16 = mybir.dt.bfloat16
    npb = ntpb * P
    nst = -(-ntpb // ST)          # supertiles per block
    Sig = mybir.ActivationFunctionType.Sigmoid
    EQ = mybir.AluOpType.is_equal
    Wg = VG * VT
    ngrp = vpad // Wg
    do1 = phase in ("both", "p1")
    do2 = phase in ("both", "p2")

    # phase 1 (attention + segment reduce) and phase 2 (vocab scoring) are
    # software-pipelined: block b+1's phase-1 work is emitted in per-supertile
    # chunks between block b's phase-2 vocab groups, so the in-order engines
    # never expose phase-1 latency at block boundaries and the y-store stream
    # stays saturated.
    with tc.tile_pool(name="p1big", bufs=2) as pb, \
         tc.tile_pool(name="p1", bufs=3) as pool, \
         tc.tile_pool(name="blkp", bufs=2) as blkp, \
         tc.tile_pool(name="p2o", bufs=8) as p2o, \
         tc.tile_pool(name="psPre", bufs=2, space="PSUM") as psPre, \
         tc.tile_pool(name="psBc", bufs=1, space="PSUM") as psBc, \
         tc.tile_pool(name="psS", bufs=1, space="PSUM") as psS, \
         tc.tile_pool(name="psAl", bufs=1, space="PSUM") as psAl, \
         tc.tile_pool(name="ps2", bufs=3, space="PSUM") as ps2:

        def p1_segments(blk):
            """Emit phase 1 for `blk` as a list of closures (prologue,
            one per supertile, epilogue) sharing state via `sv`."""
            gsl = slice(blk * P, (blk + 1) * P)
            nsl = slice(blk * npb, (blk + 1) * npb)
            sv = {}
            n_mm = sum(min(ST, ntpb - ST * s) for s in range(nst))

            def prologue():
                sv["psS_t"] = psS.tile([P, 4 * P], f32, tag="psS", name="psS_t",
                                       space="PSUM")
                xpk = pb.tile([P, npb], bf16, tag="xpk")
                nc.sync.dma_start(out=xpk[:], in_=xpk_ext[:, nsl])
                xtp = pb.tile([P, npb], bf16, tag="xtp")
                nc.sync.dma_start(out=xtp[:], in_=xtp_ext[:, nsl])
                sv["xpk"], sv["xtp"] = xpk, xtp
                # q1g[g, d] = (v_n_blk @ W1_w.T)[g, d] (biases folded later)
                q1g_ps = sv["psS_t"][:, 0:P]
                nc.tensor.matmul(out=q1g_ps, lhsT=vnt[:, gsl], rhs=w1t[:],
                                 start=True, stop=True)
                q1g = blkp.tile([P, P], bf16, tag="q1g")
                nc.vector.tensor_copy(out=q1g[:], in_=q1g_ps)
                sv["q1g"] = q1g
                sv["mm_i"] = 0

            def supertile(st):
                blc = blc_sb[blk]
                q1g, xpk, xtp = sv["q1g"], sv["xpk"], sv["xtp"]
                sg_ps = sv["psS_t"][:, P:2 * P]
                nsub = min(ST, ntpb - ST * st)
                w = nsub * P
                ssl = slice(st * ST * P, st * ST * P + w)  # cols in block
                # batchloc replicated down partitions via K=4 row-select
                bc_ps = psBc.tile([P, ST * P], f32, tag="bc", space="PSUM")
                nc.tensor.matmul(out=bc_ps[:, :w],
                                 lhsT=sel4[:, blk * P:(blk + 1) * P],
                                 rhs=blr4[:, ssl], start=True, stop=True)
                # S^T[g, n] = (batchloc[n] == g)   [one op, 512 wide]
                StT = pool.tile([P, ST * P], bf16, tag="StT")
                nc.vector.tensor_scalar(out=StT[:, :w], in0=bc_ps[:, :w],
                                        scalar1=iota_col[:], scalar2=None,
                                        op0=EQ)
                # S[n, g] per 128-node subtile
                S_st = pool.tile([P, ST * P], bf16, tag="S")
                for c in range(nsub):
                    csl = slice(c * P, (c + 1) * P)
                    nc.vector.tensor_scalar(
                        out=S_st[:, csl], in0=iota_row[:],
                        scalar1=blc[:, st * ST + c:st * ST + c + 1],
                        scalar2=None, op0=EQ)
                # pre^T[d, n] = W2 @ x^T + q1g^T-expand   (+b12 in ACT)
                pre_ps = psPre.tile([P, ST * P], f32, tag="pre", space="PSUM")
                nc.tensor.matmul(out=pre_ps[:, :w], lhsT=w2t[:],
                                 rhs=xtp[:, ssl], start=True, stop=False)
                nc.tensor.matmul(out=pre_ps[:, :w], lhsT=q1g[:],
                                 rhs=StT[:, :w], start=False, stop=True)
                sigT = pool.tile([P, ST * P], bf16, tag="sigT")
                nc.scalar.activation(out=sigT[:, :w], in_=pre_ps[:, :w],
                                     func=Sig, bias=b12c[:])
                # alpha[n] = sig @ q_w.T (+ q_b in copy)
                al_ps = psAl.tile([P, ST], f32, tag="al", space="PSUM")
                for c in range(nsub):
                    csl = slice(c * P, (c + 1) * P)
                    nc.tensor.matmul(out=al_ps[:, c:c + 1],
                                     lhsT=sigT[:, csl], rhs=qwt[:],
                                     start=True, stop=True)
                al = pool.tile([P, ST], f32, tag="al")
                nc.vector.tensor_scalar_add(out=al[:, :nsub],
                                            in0=al_ps[:, :nsub],
                                            scalar1=qbc[:])
                # xa = alpha * x ; s_g^T[d, g] += xa^T-reduce via S
                xa = pool.tile([P, ST * P], bf16, tag="xa")
                for c in range(nsub):
                    csl = slice(c * P, (c + 1) * P)
                    nc.vector.tensor_scalar_mul(
                        out=xa[:, csl],
                        in0=xpk[:, st * ST * P + c * P:st * ST * P + (c + 1) * P],
                        scalar1=al[:, c:c + 1])
                    nc.tensor.matmul(out=sg_ps, lhsT=xa[:, csl],
                                     rhs=S_st[:, csl],
                                     start=(sv["mm_i"] == 0),
                                     stop=(sv["mm_i"] == n_mm - 1))
                    sv["mm_i"] += 1

            def epilogue():
                sg_ps = sv["psS_t"][:, P:2 * P]
                sg_sb = blkp.tile([P, P], f32, tag="sgsb")
                nc.vector.tensor_copy(out=sg_sb[:], in_=sg_ps)
                # s_h^T[d, g] = W3a @ v_n^T + W3b @ s_g^T  (+W3_b in copy)
                sh_ps = sv["psS_t"][:, 0:P]
                nc.tensor.matmul(out=sh_ps, lhsT=w3at[:], rhs=vnt[:, gsl],
                                 start=True, stop=False)
                nc.tensor.matmul(out=sh_ps, lhsT=w3bt[:], rhs=sg_sb[:],
                                 start=False, stop=True)
                nc.vector.tensor_scalar_add(out=shT[:, gsl], in0=sh_ps,
                                            scalar1=w3bc[:])

            return ([prologue]
                    + [lambda st=st: supertile(st) for st in range(nst)]
                    + [epilogue])

        def p2_group(blk, g):
            gsl = slice(blk * P, (blk + 1) * P)
            gvsl = slice(g * Wg, (g + 1) * Wg)
            itw = itw_pre[g]
            sc = p2o.tile([P, Wg], bf16, tag="scsb")
            for s in range(VG):
                sc_ps = ps2.tile([P, VT], f32, tag="sc", space="PSUM")
                nc.tensor.matmul(out=sc_ps[:],
                                 lhsT=shT[:, gsl],
                                 rhs=itw[:, s * VT:(s + 1) * VT],
                                 start=True, stop=True)
                if (g * VG + s) % 8 < 3:
                    nc.vector.tensor_copy(out=sc[:, s * VT:(s + 1) * VT],
                                          in_=sc_ps[:])
                else:
                    nc.scalar.copy(out=sc[:, s * VT:(s + 1) * VT],
                                   in_=sc_ps[:])
            nc.sync.dma_start(out=y_ext[blk * P:(blk + 1) * P, gvsl],
                              in_=sc[:])

        if not do2:
            rep_ctx = (tc.For_i(0, repeat, 1) if repeat > 1
                       else contextlib.nullcontext())
            with rep_ctx:
                for blk in range(NBLK):
                    for seg in p1_segments(blk):
                        seg()
            return
        if do1:
            # prologue: block 0's phase 1 runs once before the loop; inside
            # the loop each iteration computes block 0's (identical) phase 1
            # for the NEXT iteration under block 3's store stream.
            for seg in p1_segments(0):
                seg()
        rep_ctx = (tc.For_i(0, repeat, 1) if repeat > 1
                   else contextlib.nullcontext())
        with rep_ctx:
            for blk in range(NBLK):
                nxt = list(p1_segments((blk + 1) % NBLK)) if do1 else []
                for g in range(ngrp):
                    p2_group(blk, g)
                    if nxt and g >= 1:
                        nxt.pop(0)()
                        if len(nxt) > ngrp - g - 2:   # don't leave a tail
                            nxt.pop(0)()
                while nxt:
                    nxt.pop(0)()


def prep_inputs(session_embedding, item_weight, W1_w, W1_b, W2_w, W2_b,
                q_w, q_b, W3_w, W3_b, batch, num_graphs):
    """Host-side sharding/layout. Returns (in_maps, ntpb, vpad, V, cc)."""
    x = np.ascontiguousarray(np.asarray(session_embedding, dtype=np.float32))
    itw = np.asarray(item_weight, dtype=np.float32)
    batch = np.asarray(batch).astype(np.int64)
    B = int(num_graphs)
    N, d = x.shape
    V = itw.shape[0]
    assert d == D and B == NCORES * BC, (d, B)

    counts = np.bincount(batch, minlength=B)
    assert counts.min() >= 1, "every graph must be non-empty"
    starts = np.zeros(B + 1, np.int64)
    np.cumsum(counts, out=starts[1:])
    assert starts[-1] == N
    last_idx = starts[1:] - 1
    v_n = x[last_idx]                                   # [B, D]

    blk_cnt = starts[P::P] - starts[:-P:P].reshape(-1)  # [B//P]
    ntpb = int(-(-blk_cnt.max() // P))                  # ceil
    npb = ntpb * P

    vpad = -(-V // (VT * VG)) * (VT * VG)
    itwT = np.zeros((D, vpad), BF16)
    itwT[:, :V] = itw.T.astype(BF16)

    w1t = np.ascontiguousarray(np.asarray(W1_w, np.float32).T)
    w2t = np.ascontiguousarray(np.asarray(W2_w, np.float32).T.astype(BF16))
    W3 = np.asarray(W3_w, np.float32)
    w3at = np.ascontiguousarray(W3[:, :D].T)
    w3bt = np.ascontiguousarray(W3[:, D:].T)
    b12c = (np.asarray(W1_b, np.float32) + np.asarray(W2_b, np.float32)
            ).reshape(P, 1).copy()
    w3bc = np.asarray(W3_b, np.float32).reshape(P, 1).copy()
    qwt = np.ascontiguousarray(
        np.asarray(q_w, np.float32).reshape(1, D).T.astype(BF16))
    sel4 = np.zeros((NBLK, NBLK * P), BF16)
    for k in range(NBLK):
        sel4[k, k * P:(k + 1) * P] = 1
    qbc = np.full((P, 1), np.float32(np.asarray(q_b).reshape(())), np.float32)

    in_maps = []
    for c in range(NCORES):
        xpad = np.zeros((NBLK, npb, D), np.float32)
        bl = np.zeros((NBLK, P, ntpb), np.float32)
        blr = np.zeros((NBLK, npb), BF16)
        for b in range(NBLK):
            glo = c * BC + b * P
            s, e = int(starts[glo]), int(starts[glo + P])
            cnt = e - s
            assert cnt <= npb
            xpad[b, :cnt] = x[s:e]
            locp = np.zeros(npb, np.float32)
            locp[:cnt] = (batch[s:e] - glo).astype(np.float32)
            bl[b] = locp.reshape(ntpb, P).T
            blr[b] = locp.astype(BF16)
        # packed node-row: xpk[:, blk*npb + t*128 + j][i] = x_pad[blk, t*128+i, j]
        xpk = np.ascontiguousarray(
            xpad.reshape(NBLK, ntpb, P, D).transpose(2, 0, 1, 3)
            .reshape(P, NBLK * npb).astype(BF16))
        # feature-row transposed: xtp[:, blk*npb + n] = x_pad[blk, n, :]
        xtp = np.ascontiguousarray(
            xpad.transpose(2, 0, 1).reshape(P, NBLK * npb).astype(BF16))
        vnt = np.ascontiguousarray(v_n[c * BC:(c + 1) * BC].T)
        im = dict(
            xpk=xpk, xtp=xtp, blc=np.ascontiguousarray(bl),
            blr=np.ascontiguousarray(blr), vnt=vnt,
            w1t=w1t, w2t=w2t, w3at=w3at, w3bt=w3bt,
            b12c=b12c, w3bc=w3bc, qwt=qwt, qbc=qbc, sel4=sel4, itwt=itwT)
        in_maps.append(im)
    return in_maps, ntpb, vpad, V, False


_NC_CACHE = {}


def get_nc(ntpb, vpad, repeat=1, phase="both", cc=False):
    key = (ntpb, vpad, repeat, phase)
    if key not in _NC_CACHE:
        _NC_CACHE[key] = build_nc(ntpb, vpad, repeat, phase)
    return _NC_CACHE[key]


def kernel(**inputs):
    from concourse.bass_utils import run_bass_kernel_spmd

    in_maps, ntpb, vpad, V, cc = prep_inputs(**inputs)
    nc = get_nc(ntpb, vpad)
    res = run_bass_kernel_spmd(nc, in_maps, core_ids=list(range(NCORES)))
    B = int(inputs["num_graphs"])
    y = np.empty((B, V), np.float32)
    for c in range(NCORES):
        y[c * BC:(c + 1) * BC] = res.results[c]["y"][:, :V]
    return y


# revision 23
# speedup vs baseline: 3.3574x; 1.1123x over previous
"""Trainium2 Bass kernel for nn_Embedding2Score (segment_reduce).

Strategy (data-parallel over sessions, per sharding hint):
  - 4096 graphs -> 8 cores x 512 graphs (4 blocks of 128 graphs each).
    Each core owns whole contiguous segments (batch is sorted by graph).
  - Nodes are processed in 512-node supertiles (4 x 128-node subtiles).
    Segment broadcast (v_n -> nodes) and segment sum (alpha*x -> s_g) are
    one-hot matmuls on PE; one-hot blocks are built with DVE is_equal.
    The graph-major one-hot S^T needs batchloc replicated down the 128
    partitions: done with a K=1 ones-matmul into PSUM (no DMA broadcast).
  - bf16 data path: x (both layouts), item_weight, W2, q_w and all one-hot
    matrices are bf16 (PE runs 1 cycle/row vs 4 for fp32, DMA bytes halve).
    Accumulation stays fp32 in PSUM; biases/alpha/v_n/W1/W3 stay fp32.
    rel-err budget is 2e-2; bf16 keeps it ~2e-3.
  - Final scoring: s_h^T [128d, 512g] per core vs item_weight^T tiles,
    grouped 4 vocab-tiles per DMA so all phase-2 DMAs are ~1 MB.
    Output rows [512, V] per core = row-slice of [4096, 50000].
"""

import sys

if "/opt/trn_rl_repo" not in sys.path:
    sys.path.insert(0, "/opt/trn_rl_repo")

import numpy as np
import ml_dtypes

BF16 = ml_dtypes.bfloat16
P = 128          # partitions / tile edge
D = 128          # hidden size
NCORES = 8
NBLK = 4         # graph blocks per core, 128 graphs each
BC = NBLK * P    # graphs per core = 512
VT = 500         # vocab tile (psum bank holds 512 fp32)
VG = 4           # vocab tiles per DMA group
ST = 4           # 128-node subtiles per supertile
NPF = 25         # vocab groups prefetched (all SBUF-resident)


def build_nc(ntpb, vpad, repeat=1, phase="both"):
    """Build the per-core Bass program. ntpb = node tiles per graph-block,
    vpad = padded vocab size (multiple of VT*VG). repeat>1 wraps the body
    in a hardware loop (timing probes). phase: 'both' | 'p1' | 'p2'."""
    import contextlib
    import concourse.bacc as bacc
    import concourse.mybir as mybir
    from concourse.tile import TileContext

    f32 = mybir.dt.float32
    bf16 = mybir.dt.bfloat16
    npb = ntpb * P
    nc = bacc.Bacc()

    xpk_ext = nc.declare_dram_parameter("xpk", [P, NBLK * npb], bf16, isOutput=False)
    xtp_ext = nc.declare_dram_parameter("xtp", [P, NBLK * npb], bf16, isOutput=False)
    blc_ext = nc.declare_dram_parameter("blc", [NBLK, P, ntpb], f32, isOutput=False)
    blr_ext = nc.declare_dram_parameter("blr", [NBLK, npb], bf16, isOutput=False)
    vnt_ext = nc.declare_dram_parameter("vnt", [D, BC], f32, isOutput=False)
    w1t_ext = nc.declare_dram_parameter("w1t", [D, D], f32, isOutput=False)
    w2t_ext = nc.declare_dram_parameter("w2t", [D, D], bf16, isOutput=False)
    w3at_ext = nc.declare_dram_parameter("w3at", [D, D], f32, isOutput=False)
    w3bt_ext = nc.declare_dram_parameter("w3bt", [D, D], f32, isOutput=False)
    b12c_ext = nc.declare_dram_parameter("b12c", [P, 1], f32, isOutput=False)
    w3bc_ext = nc.declare_dram_parameter("w3bc", [P, 1], f32, isOutput=False)
    qwt_ext = nc.declare_dram_parameter("qwt", [D, 1], bf16, isOutput=False)
    sel4_ext = nc.declare_dram_parameter("sel4", [NBLK, NBLK * P], bf16,
                                         isOutput=False)
    qbc_ext = nc.declare_dram_parameter("qbc", [P, 1], f32, isOutput=False)
    itwt_ext = nc.declare_dram_parameter("itwt", [D, vpad], bf16, isOutput=False)
    y_ext = nc.declare_dram_parameter("y", [BC, vpad], bf16, isOutput=True)

    with TileContext(nc) as tc:
        with tc.tile_pool(name="const", bufs=1) as cp:
            iota_i = cp.tile([P, P], mybir.dt.int32, tag="iotai")
            nc.gpsimd.iota(iota_i[:], pattern=[[1, P]], base=0, channel_multiplier=0)
            iota_f = cp.tile([P, P], f32, tag="iotaf")
            nc.vector.tensor_copy(out=iota_f[:], in_=iota_i[:])
            iota_row = cp.tile([P, P], bf16, tag="iotarow")
            nc.vector.tensor_copy(out=iota_row[:], in_=iota_f[:])
            iota_ci = cp.tile([P, 1], mybir.dt.int32, tag="iotaci")
            nc.gpsimd.iota(iota_ci[:], pattern=[[0, 1]], base=0, channel_multiplier=1)
            iota_col = cp.tile([P, 1], f32, tag="iotacol")
            nc.vector.tensor_copy(out=iota_col[:], in_=iota_ci[:])
            ones_row = cp.tile([1, P], bf16, tag="onesrow")
            nc.vector.memset(ones_row[:], 1.0)

            def load(name, ext, shape, dt):
                t = cp.tile(shape, dt, tag=name)
                nc.sync.dma_start(out=t[:], in_=ext[:])
                return t

            w1t = load("w1t", w1t_ext, [D, D], f32)
            w2t = load("w2t", w2t_ext, [D, D], bf16)
            w3at = load("w3at", w3at_ext, [D, D], f32)
            w3bt = load("w3bt", w3bt_ext, [D, D], f32)
            b12c = load("b12c", b12c_ext, [P, 1], f32)
            w3bc = load("w3bc", w3bc_ext, [P, 1], f32)
            qwt = load("qwt", qwt_ext, [D, 1], bf16)
            qbc = load("qbc", qbc_ext, [P, 1], f32)
            vnt = load("vnt", vnt_ext, [D, BC], f32)

            shT = cp.tile([D, BC], bf16, tag="shT")  # s_h^T, filled per block
            if phase == "p2":
                nc.vector.memset(shT[:], 0.01)
            # per-block batchloc tables are iteration-invariant: load once
            blc_sb = []
            for blk in range(NBLK):
                t = cp.tile([P, ntpb], f32, tag=f"blc{blk}")
                nc.sync.dma_start(out=t[:], in_=blc_ext[blk])
                blc_sb.append(t)
            blr4 = cp.tile([NBLK, npb], bf16, tag="blr4")
            nc.sync.dma_start(out=blr4[:], in_=blr_ext[:])
            sel4 = cp.tile([NBLK, NBLK * P], bf16, tag="sel4")
            nc.sync.dma_start(out=sel4[:], in_=sel4_ext[:])
            itw_pre = []
            W = VG * VT
            for g in range(min(NPF, vpad // W)):
                t = cp.tile([D, W], bf16, tag=f"itwpre{g}")
                nc.sync.dma_start(out=t[:], in_=itwt_ext[:, g * W:(g + 1) * W])
                itw_pre.append(t)

            _build_body(nc, tc, mybir, ntpb, vpad,
                        xpk_ext, xtp_ext, blc_sb, blr4, sel4, itwt_ext, y_ext,
                        iota_row, iota_col, ones_row,
                        w1t, w2t, w3at, w3bt, b12c, w3bc, qwt, qbc, vnt, shT,
                        phase, itw_pre, repeat)

    nc.compile()
    return nc


def _build_body(nc, tc, mybir, ntpb, vpad,
                xpk_ext, xtp_ext, blc_sb, blr4, sel4, itwt_ext, y_ext,
                iota_row, iota_col, ones_row,
                w1t, w2t, w3at, w3bt, b12c, w3bc, qwt, qbc, vnt, shT,
                phase="both", itw_pre=None, repeat=1):
    import contextlib
    f32 = mybir.dt.float32
    bf16 = mybir.dt.bfloat16
    npb = ntpb * P
    nst = -(-ntpb // ST)          # supertiles per block
    Sig = mybir.ActivationFunctionType.Sigmoid
    EQ = mybir.AluOpType.is_equal
    Wg = VG * VT
    ngrp = vpad // Wg
    do1 = phase in ("both", "p1")
    do2 = phase in ("both", "p2")

    # phase 1 (attention + segment reduce) and phase 2 (vocab scoring) are
    # software-pipelined: block b+1's phase-1 work is emitted in per-supertile
    # chunks between block b's phase-2 vocab groups, so the in-order engines
    # never expose phase-1 latency at block boundaries and the y-store stream
    # stays saturated.
    with tc.tile_pool(name="p1big", bufs=2) as pb, \
         tc.tile_pool(name="p1", bufs=3) as pool, \
         tc.tile_pool(name="blkp", bufs=2) as blkp, \
         tc.tile_pool(name="p2o", bufs=8) as p2o, \
         tc.tile_pool(name="psPre", bufs=2, space="PSUM") as psPre, \
         tc.tile_pool(name="psBc", bufs=1, space="PSUM") as psBc, \
         tc.tile_pool(name="psS", bufs=1, space="PSUM") as psS, \
         tc.tile_pool(name="psAl", bufs=1, space="PSUM") as psAl, \
         tc.tile_pool(name="ps2", bufs=3, space="PSUM") as ps2:

        def p1_segments(blk):
            """Emit phase 1 for `blk` as a list of closures (prologue,
            one per supertile, epilogue) sharing state via `sv`."""
            gsl = slice(blk * P, (blk + 1) * P)
            nsl = slice(blk * npb, (blk + 1) * npb)
            sv = {}
            n_mm = sum(min(ST, ntpb - ST * s) for s in range(nst))

            def prologue():
                sv["psS_t"] = psS.tile([P, 4 * P], f32, tag="psS", name="psS_t",
                                       space="PSUM")
                xpk = pb.tile([P, npb], bf16, tag="xpk")
                nc.sync.dma_start(out=xpk[:], in_=xpk_ext[:, nsl])
                xtp = pb.tile([P, npb], bf16, tag="xtp")
                nc.sync.dma_start(out=xtp[:], in_=xtp_ext[:, nsl])
                sv["xpk"], sv["xtp"] = xpk, xtp
                # q1g[g, d] = (v_n_blk @ W1_w.T)[g, d] (biases folded later)
                q1g_ps = sv["psS_t"][:, 0:P]
                nc.tensor.matmul(out=q1g_ps, lhsT=vnt[:, gsl], rhs=w1t[:],
                                 start=True, stop=True)
                q1g = blkp.tile([P, P], bf16, tag="q1g")
                nc.vector.tensor_copy(out=q1g[:], in_=q1g_ps)
                sv["q1g"] = q1g
                sv["mm_i"] = 0

            def supertile(st):
                blc = blc_sb[blk]
                q1g, xpk, xtp = sv["q1g"], sv["xpk"], sv["xtp"]
                sg_ps = sv["psS_t"][:, P:2 * P]
                nsub = min(ST, ntpb - ST * st)
                w = nsub * P
                ssl = slice(st * ST * P, st * ST * P + w)  # cols in block
                # batchloc replicated down partitions via K=4 row-select
                bc_ps = psBc.tile([P, ST * P], f32, tag="bc", space="PSUM")
                nc.tensor.matmul(out=bc_ps[:, :w],
                                 lhsT=sel4[:, blk * P:(blk + 1) * P],
                                 rhs=blr4[:, ssl], start=True, stop=True)
                # S^T[g, n] = (batchloc[n] == g)   [one op, 512 wide]
                StT = pool.tile([P, ST * P], bf16, tag="StT")
                nc.vector.tensor_scalar(out=StT[:, :w], in0=bc_ps[:, :w],
                                        scalar1=iota_col[:], scalar2=None,
                                        op0=EQ)
                # S[n, g] for the whole supertile in one op: compare the
                # tiled iota against per-subtile graph ids broadcast 128-wide
                S_st = pool.tile([P, ST * P], bf16, tag="S")
                nc.vector.tensor_tensor(
                    out=S_st[:, :w].rearrange("p (c g) -> p c g", c=nsub),
                    in0=iota_row[:].unsqueeze(1).to_broadcast((P, nsub, P)),
                    in1=blc[:, st * ST:st * ST + nsub].unsqueeze(2)
                        .to_broadcast((P, nsub, P)),
                    op=EQ)
                # pre^T[d, n] = W2 @ x^T + q1g^T-expand   (+b12 in ACT)
                pre_ps = psPre.tile([P, ST * P], f32, tag="pre", space="PSUM")
                nc.tensor.matmul(out=pre_ps[:, :w], lhsT=w2t[:],
                                 rhs=xtp[:, ssl], start=True, stop=False)
                nc.tensor.matmul(out=pre_ps[:, :w], lhsT=q1g[:],
                                 rhs=StT[:, :w], start=False, stop=True)
                sigT = pool.tile([P, ST * P], bf16, tag="sigT")
                nc.scalar.activation(out=sigT[:, :w], in_=pre_ps[:, :w],
                                     func=Sig, bias=b12c[:])
                # alpha[n] = sig @ q_w.T (+ q_b in copy)
                al_ps = psAl.tile([P, ST], f32, tag="al", space="PSUM")
                for c in range(nsub):
                    csl = slice(c * P, (c + 1) * P)
                    nc.tensor.matmul(out=al_ps[:, c:c + 1],
                                     lhsT=sigT[:, csl], rhs=qwt[:],
                                     start=True, stop=True)
                al = pool.tile([P, ST], f32, tag="al")
                nc.vector.tensor_scalar_add(out=al[:, :nsub],
                                            in0=al_ps[:, :nsub],
                                            scalar1=qbc[:])
                # xa = alpha * x ; s_g^T[d, g] += xa^T-reduce via S
                xa = pool.tile([P, ST * P], bf16, tag="xa")
                nc.vector.tensor_tensor(
                    out=xa[:, :w].rearrange("p (c n) -> p c n", c=nsub),
                    in0=xpk[:, ssl].rearrange("p (c n) -> p c n", c=nsub),
                    in1=al[:, :nsub].unsqueeze(2).to_broadcast((P, nsub, P)),
                    op=mybir.AluOpType.mult)
                for c in range(nsub):
                    csl = slice(c * P, (c + 1) * P)
                    nc.tensor.matmul(out=sg_ps, lhsT=xa[:, csl],
                                     rhs=S_st[:, csl],
                                     start=(sv["mm_i"] == 0),
                                     stop=(sv["mm_i"] == n_mm - 1))
                    sv["mm_i"] += 1

            def epilogue():
                sg_ps = sv["psS_t"][:, P:2 * P]
                sg_sb = blkp.tile([P, P], f32, tag="sgsb")
                nc.vector.tensor_copy(out=sg_sb[:], in_=sg_ps)
                # s_h^T[d, g] = W3a @ v_n^T + W3b @ s_g^T  (+W3_b in copy)
                sh_ps = sv["psS_t"][:, 0:P]
                nc.tensor.matmul(out=sh_ps, lhsT=w3at[:], rhs=vnt[:, gsl],
                                 start=True, stop=False)
                nc.tensor.matmul(out=sh_ps, lhsT=w3bt[:], rhs=sg_sb[:],
                                 start=False, stop=True)
                nc.vector.tensor_scalar_add(out=shT[:, gsl], in0=sh_ps,
                                            scalar1=w3bc[:])

            return ([prologue]
                    + [lambda st=st: supertile(st) for st in range(nst)]
                    + [epilogue])

        def p2_group(blk, g):
            gsl = slice(blk * P, (blk + 1) * P)
            gvsl = slice(g * Wg, (g + 1) * Wg)
            itw = itw_pre[g]
            sc = p2o.tile([P, Wg], bf16, tag="scsb")
            for s in range(VG):
                sc_ps = ps2.tile([P, VT], f32, tag="sc", space="PSUM")
                nc.tensor.matmul(out=sc_ps[:],
                                 lhsT=shT[:, gsl],
                                 rhs=itw[:, s * VT:(s + 1) * VT],
                                 start=True, stop=True)
                if (g * VG + s) % 8 < 3:
                    nc.vector.tensor_copy(out=sc[:, s * VT:(s + 1) * VT],
                                          in_=sc_ps[:])
                else:
                    nc.scalar.copy(out=sc[:, s * VT:(s + 1) * VT],
                                   in_=sc_ps[:])
            nc.sync.dma_start(out=y_ext[blk * P:(blk + 1) * P, gvsl],
                              in_=sc[:])

        if not do2:
            rep_ctx = (tc.For_i(0, repeat, 1) if repeat > 1
                       else contextlib.nullcontext())
            with rep_ctx:
                for blk in range(NBLK):
                    for seg in p1_segments(blk):
                        seg()
            return
        if do1:
            # prologue: block 0's phase 1 runs once before the loop; inside
            # the loop each iteration computes block 0's (identical) phase 1
            # for the NEXT iteration under block 3's store stream.
            for seg in p1_segments(0):
                seg()
        rep_ctx = (tc.For_i(0, repeat, 1) if repeat > 1
                   else contextlib.nullcontext())
        with rep_ctx:
            for blk in range(NBLK):
                nxt = list(p1_segments((blk + 1) % NBLK)) if do1 else []
                for g in range(ngrp):
                    p2_group(blk, g)
                    if nxt and g >= 1 and g % 2 == 1:
                        nxt.pop(0)()
                    if nxt and len(nxt) > ngrp - g - 2:  # don't leave a tail
                        nxt.pop(0)()
                while nxt:
                    nxt.pop(0)()


def prep_inputs(session_embedding, item_weight, W1_w, W1_b, W2_w, W2_b,
                q_w, q_b, W3_w, W3_b, batch, num_graphs):
    """Host-side sharding/layout. Returns (in_maps, ntpb, vpad, V, cc)."""
    x = np.ascontiguousarray(np.asarray(session_embedding, dtype=np.float32))
    itw = np.asarray(item_weight, dtype=np.float32)
    batch = np.asarray(batch).astype(np.int64)
    B = int(num_graphs)
    N, d = x.shape
    V = itw.shape[0]
    assert d == D and B == NCORES * BC, (d, B)

    counts = np.bincount(batch, minlength=B)
    assert counts.min() >= 1, "every graph must be non-empty"
    starts = np.zeros(B + 1, np.int64)
    np.cumsum(counts, out=starts[1:])
    assert starts[-1] == N
    last_idx = starts[1:] - 1
    v_n = x[last_idx]                                   # [B, D]

    blk_cnt = starts[P::P] - starts[:-P:P].reshape(-1)  # [B//P]
    ntpb = int(-(-blk_cnt.max() // P))                  # ceil
    npb = ntpb * P

    vpad = -(-V // (VT * VG)) * (VT * VG)
    itwT = np.zeros((D, vpad), BF16)
    itwT[:, :V] = itw.T.astype(BF16)

    w1t = np.ascontiguousarray(np.asarray(W1_w, np.float32).T)
    w2t = np.ascontiguousarray(np.asarray(W2_w, np.float32).T.astype(BF16))
    W3 = np.asarray(W3_w, np.float32)
    w3at = np.ascontiguousarray(W3[:, :D].T)
    w3bt = np.ascontiguousarray(W3[:, D:].T)
    b12c = (np.asarray(W1_b, np.float32) + np.asarray(W2_b, np.float32)
            ).reshape(P, 1).copy()
    w3bc = np.asarray(W3_b, np.float32).reshape(P, 1).copy()
    qwt = np.ascontiguousarray(
        np.asarray(q_w, np.float32).reshape(1, D).T.astype(BF16))
    sel4 = np.zeros((NBLK, NBLK * P), BF16)
    for k in range(NBLK):
        sel4[k, k * P:(k + 1) * P] = 1
    qbc = np.full((P, 1), np.float32(np.asarray(q_b).reshape(())), np.float32)

    in_maps = []
    for c in range(NCORES):
        xpad = np.zeros((NBLK, npb, D), np.float32)
        bl = np.zeros((NBLK, P, ntpb), np.float32)
        blr = np.zeros((NBLK, npb), BF16)
        for b in range(NBLK):
            glo = c * BC + b * P
            s, e = int(starts[glo]), int(starts[glo + P])
            cnt = e - s
            assert cnt <= npb
            xpad[b, :cnt] = x[s:e]
            locp = np.zeros(npb, np.float32)
            locp[:cnt] = (batch[s:e] - glo).astype(np.float32)
            bl[b] = locp.reshape(ntpb, P).T
            blr[b] = locp.astype(BF16)
        # packed node-row: xpk[:, blk*npb + t*128 + j][i] = x_pad[blk, t*128+i, j]
        xpk = np.ascontiguousarray(
            xpad.reshape(NBLK, ntpb, P, D).transpose(2, 0, 1, 3)
            .reshape(P, NBLK * npb).astype(BF16))
        # feature-row transposed: xtp[:, blk*npb + n] = x_pad[blk, n, :]
        xtp = np.ascontiguousarray(
            xpad.transpose(2, 0, 1).reshape(P, NBLK * npb).astype(BF16))
        vnt = np.ascontiguousarray(v_n[c * BC:(c + 1) * BC].T)
        im = dict(
            xpk=xpk, xtp=xtp, blc=np.ascontiguousarray(bl),
            blr=np.ascontiguousarray(blr), vnt=vnt,
            w1t=w1t, w2t=w2t, w3at=w3at, w3bt=w3bt,
            b12c=b12c, w3bc=w3bc, qwt=qwt, qbc=qbc, sel4=sel4, itwt=itwT)
        in_maps.append(im)
    return in_maps, ntpb, vpad, V, False


_NC_CACHE = {}


def get_nc(ntpb, vpad, repeat=1, phase="both", cc=False):
    key = (ntpb, vpad, repeat, phase)
    if key not in _NC_CACHE:
        _NC_CACHE[key] = build_nc(ntpb, vpad, repeat, phase)
    return _NC_CACHE[key]


def kernel(**inputs):
    from concourse.bass_utils import run_bass_kernel_spmd

    in_maps, ntpb, vpad, V, cc = prep_inputs(**inputs)
    nc = get_nc(ntpb, vpad)
    res = run_bass_kernel_spmd(nc, in_maps, core_ids=list(range(NCORES)))
    B = int(inputs["num_graphs"])
    y = np.empty((B, V), np.float32)
    for c in range(NCORES):
        y[c * BC:(c + 1) * BC] = res.results[c]["y"][:, :V]
    return y
